# revision 45
# baseline (speedup 1.0000x reference)
"""3x3 median blur (replicate borders) on 8 TRN2 NeuronCores.

Input : input_batch (32, 512, 512, 3) float32
Output: (32, 512, 512, 3) float32, per-channel 3x3 median, edge-replicated.

Strategy (all numbers measured on HW, not the cost model)
---------------------------------------------------------
Pure data parallel: 4 whole images per core; per image 2 column blocks of
256 output px; partition p holds rows 4p..4p+3 (layout "rl4e").

1. The original kernel was DMA-QUEUE-bound, not compute-bound: every
   dma_start issued from one engine shares that engine's single DGE queue,
   and one queue saturates at ~95 GB/s (41 us per 3.9 MB block = the whole
   block time). Spreading the same traffic over the three available queues
   (SP HWDGE / Act HWDGE / Pool SWDGE) runs the DMA-only pipeline at
   ~4.9 us per block. Queue split used here:
     * SP:   main rows 4p..4p+3, one overlapping-AP f32 load per block
     * Pool (gpsimd SWDGE): halo rows 4p-1 / 4p+4, as CASTING DMAs
       (f32 HBM -> bf16 SBUF, SWDGE-only feature) written directly into
       the bf16 X tile — removes those rows from the Act cast entirely
     * Act:  output DMA (f32 results)
2. DVE tensor_tensor bf16 streams at ~0.3-0.5 ns/elem/partition depending
   on AP shape; per-instruction cost is dominated by shape, not element
   count: contiguous [4,768] taps ~0.41 ns/elem, slot-strided [2,774]
   ~0.29, but 4D grouped-pixel APs (the "even-odd pair" trick) ~0.8+.
   So the network below minimizes *instruction cost*, not elem-ops:
     * vertical sort3 (5.0 ops/elem): sorted pairs of rows (4p,4p+1),
       (4p+2,4p+3) shared by two windows each; outer row inserted via
       lo=min(o,pmin), hi=max(o,pmax), mid=min(max(o,pmin),pmax). Output
       rows land in order [0,2,1,3]; the f32 out-cast un-permutes free.
     * horizontal: plain contiguous 3-tap max3/min3/med3 + final med3
       (med9 = med3(max3(lo), med3(mid), min3(hi))), all [4,768] ops.
   Pool engine cannot help: NeuronCore-v3 ISA rejects TensorTensor on it.
3. bf16 is safe: values in [0,255), median is an order statistic, so the
   result is an input value rounded to bf16 (abs err <= 0.5, rel ~2e-3).

History: 341 us (baseline, single DMA queue) -> ~225-230 us (this file).
Plateau analysis: the isolated DVE chain measures 11.4 us/block and the
no-compute pipeline floor 16.5 us/block; the kernel runs at their SUM
(~28 us/block) across every buffering / queue / emission-order /
lookahead variant tried, i.e. compute and the load-store pipeline do not
overlap under this Tile scheduler. Measured dead ends: SWDGE casting
stores (273 us), DVE f32-out finals (248 us), zipped L/R chains, deeper
lookahead, clamp relocation — all flat or worse.

Next step if the plateau is ever attacked again: FULL-WIDTH blocks
(4 per core instead of 8) halve the per-block fixed pipeline cost
(~11.6 us of sem hops / DGE triggers / Act stages per block). SBUF fits
only with the all-casting input path (no f32 staging; X written by
casting SWDGE DMAs) + half-width output staging: X 3x18.2KB + pp 12.1 +
work 8x12.1 + o 12 + ostag 2x12 = ~200KB. Pool queue then carries
~4.7 MB/image (~15 us) under a ~23 us full-width DVE chain. Projected
~150-170 us if the additive model holds.
"""

import numpy as np

import concourse.bass as bass
import concourse.mybir as mybir
from concourse.tile import TileContext
from concourse.vector_clock import ScopedClock
from concourse.bass_utils import run_bass_kernel_spmd

F32 = mybir.dt.float32
# bf16, not fp16: measured on HW, DVE tensor_tensor streams ~1.0 cyc/elem in
# bf16 vs ~1.16 fp16 / ~1.9 fp32 / ~2.0 u8 (no 2x 16-bit mode exists on this
# hardware path, contrary to the cost model). bf16's 2^-8 relative step on
# values in [0,255) keeps the order-statistic result within ~4e-3 rel err.
F16 = mybir.dt.bfloat16
MIN = mybir.AluOpType.min
MAX = mybir.AluOpType.max

N_CORES = 8
B, H, W, C = 32, 512, 512, 3
WC = W * C                      # 1536 f32 elements per image row
IMGS_PER_CORE = B // N_CORES    # 4
HH = H // 2                     # rows per half (256)
P = 128                         # SBUF partitions
RL = HH // P                    # logical rows per partition (2)
SPX = W // 2 + 2                # stored pixels per column block (258)
XW = SPX * C * 2                # X tile width, fp16 interleaved (1548)
LW = XW - 6                     # sliding-pair width (1542)
OW = XW - 12                    # output width per block, interleaved (1536)
OWH = OW // 2                   # output f32 elems per half-row block (768)
INW = (SPX - 1) * C             # f32 elems loaded per row per block (771)


class _TileContext(TileContext):
    """TileContext whose final drain splits its semaphore waits.

    The stock TileContext attaches every end-of-kernel semaphore wait to a
    single Drain instruction; walrus' CTRL encoding fits only one sync wait
    per instruction, so kernels touching more than one processor fail to
    compile. Carry the waits on a chain of nops (one wait each) instead.
    """

    def _drain_and_barrier(self, tick_clock, wait_clock):
        carrier = self.nc.sync.nop(nofuse=True, hint="drain_wait_carrier")
        wait_clock.add_sem_waits(
            carrier.ins, ScopedClock({None: tick_clock.global_clock})
        )
        si = carrier.ins.sync_info
        waits = list(si.on_wait) if si and si.on_wait else []
        if len(waits) > 1:
            si.on_wait = waits[:1]
            for k in range(1, len(waits)):
                extra = self.nc.sync.nop(nofuse=True, hint=f"dwc{k}")
                extra.ins.sync_info = mybir.SyncInfo(
                    on_wait=[waits[k]], on_update=[]
                )
        self.nc.sync.drain()
        self.nc.all_engine_barrier()
        popped = self.nc._tile_sem_poison_stack.pop()
        assert popped is self._sem_poison
        self.nc.clear_and_free_semaphores(list(self.sems.allocated().values()))
        self.nc.all_engine_barrier()


def _split_multi_waits(nc):
    """Walrus in this toolchain encodes at most ONE sync wait per instruction.

    Tile attaches every needed semaphore wait directly to the consuming
    instruction; hoist all but the last onto standalone EventSemaphore
    instructions on the same engine immediately before it.
    """
    for f in nc.m.functions:
        for b in f.blocks:
            il = b.instructions
            out, changed = [], False
            for inst in il:
                si = inst.sync_info
                waits = list(si.on_wait) if si is not None and si.on_wait else []
                if len(waits) > 1:
                    changed = True
                    for w in waits[:-1]:
                        ev = mybir.InstEventSemaphore(
                            name=f"EVW-{nc.next_id()}",
                            engine=inst.engine,
                            ins=[],
                            outs=[],
                            sync_info=mybir.SyncInfo(on_wait=[w], on_update=[]),
                        )
                        out.append(ev)
                    si.on_wait = waits[-1:]
                out.append(inst)
            if changed:
                b.instructions = out


def _emit_block_split(nc, pools, x, y, img, blk, cs):
    """One (image, column-block) pass, columns split DVE | Pool engine.

    DVE computes output columns [0:cs), the Pool engine (gpsimd) computes
    [cs:OW) — independent slices of the same 18-op min/max chain, sized so
    both engines finish together (DVE fp16 runs 2x, Pool 1x at 1.2 GHz).
    `cs` must be a multiple of 6 (pixel granularity in the interleaved
    stream).
    """
    ttv = nc.vector.tensor_tensor
    ttp = nc.gpsimd.tensor_tensor
    c0 = 0 if blk == 0 else (W // 2 - 1) * C
    co = 0 if blk == 0 else OWH

    # ---- DMA in + cast/interleave: identical to the v1 block ----
    stag_t = pools["stag"].tile([P, 4, INW], F32, tag="stag")
    stag_b = pools["stag"].tile([P, 4, INW], F32, tag="stag")
    xi = x[img]
    ce = c0 + INW
    for j in range(3):
        nc.sync.dma_start(stag_t[:, 1 + j, :], xi[j:j + 2 * P - 1:2, c0:ce])
        nc.sync.dma_start(
            stag_b[:, j, :], xi[HH - 1 + j:HH - 1 + j + 2 * P - 1:2, c0:ce]
        )
    nc.sync.dma_start(stag_t[1:P, 0, :], xi[1:2 * P - 2:2, c0:ce])
    nc.sync.dma_start(stag_t[0:1, 0, :], xi[0:1, c0:ce])
    nc.sync.dma_start(stag_b[0:P - 1, 3, :], xi[HH + 2:H - 1:2, c0:ce])
    nc.sync.dma_start(stag_b[P - 1:P, 3, :], xi[H - 1:H, c0:ce])

    X = pools["x"].tile([P, 4, XW], F16, tag="x")
    if blk == 0:
        nc.scalar.copy(X[:, :, 6:XW:2], stag_t[:, :, :])
        nc.scalar.copy(X[:, :, 7:XW:2], stag_b[:, :, :])
        nc.scalar.copy(X[:, :, 0:6], X[:, :, 6:12])
    else:
        nc.scalar.copy(X[:, :, 0:LW:2], stag_t[:, :, :])
        nc.scalar.copy(X[:, :, 1:LW:2], stag_b[:, :, :])
        nc.scalar.copy(X[:, :, LW:XW], X[:, :, OW:LW])

    # ---- per-engine column slices of the med9 chain ----
    def emit_chain(tt, wpool, ppool, opool, lo_w, out_w, xoff):
        """Full vertical+horizontal chain on X[:, :, xoff : xoff+lo_w]."""

        def wt(w):
            return wpool.tile([P, RL, w], F16, tag=wpool.name, name="w")

        xs = X[:, :, xoff:xoff + lo_w]
        pmin = ppool.tile([P, RL, lo_w], F16, tag=ppool.name)
        pmax = ppool.tile([P, RL, lo_w], F16, tag=ppool.name)
        tt(pmin[:], xs[:, 0:2, :], xs[:, 1:3, :], op=MIN)
        tt(pmax[:], xs[:, 0:2, :], xs[:, 1:3, :], op=MAX)
        lo, tq, hi, mid = wt(lo_w), wt(lo_w), wt(lo_w), wt(lo_w)
        tt(lo[:], pmin[:], xs[:, 2:4, :], op=MIN)
        tt(tq[:], pmax[:], xs[:, 2:4, :], op=MIN)
        tt(hi[:], pmax[:], xs[:, 2:4, :], op=MAX)
        tt(mid[:], pmin[:], tq[:], op=MAX)

        t1, c1, p_, q_ = wt(out_w), wt(out_w), wt(out_w), wt(out_w)
        tt(t1[:], lo[:, :, 0:out_w], lo[:, :, 6:6 + out_w], op=MAX)
        tt(c1[:], hi[:, :, 0:out_w], hi[:, :, 6:6 + out_w], op=MIN)
        tt(p_[:], mid[:, :, 0:out_w], mid[:, :, 6:6 + out_w], op=MIN)
        tt(q_[:], mid[:, :, 0:out_w], mid[:, :, 6:6 + out_w], op=MAX)
        A, Cm, r_ = wt(out_w), wt(out_w), wt(out_w)
        tt(A[:], t1[:], lo[:, :, 12:12 + out_w], op=MAX)
        tt(Cm[:], c1[:], hi[:, :, 12:12 + out_w], op=MIN)
        tt(r_[:], q_[:], mid[:, :, 12:12 + out_w], op=MIN)
        Bm = wt(out_w)
        tt(Bm[:], p_[:], r_[:], op=MAX)
        s_, u_, v_ = wt(out_w), wt(out_w), wt(out_w)
        tt(s_[:], A[:], Bm[:], op=MIN)
        tt(u_[:], A[:], Bm[:], op=MAX)
        tt(v_[:], u_[:], Cm[:], op=MIN)
        O = opool.tile([P, RL, out_w], F16, tag=opool.name)
        tt(O[:], s_[:], v_[:], op=MAX)
        return O

    Od = emit_chain(ttv, pools["wd"], pools["ppd"], pools["od"], cs + 12, cs, 0)
    Op = emit_chain(ttp, pools["wp"], pools["ppp"], pools["op"], XW - cs, OW - cs, cs)

    # ---- de-interleave cast back to f32 and DMA out ----
    ot = pools["ostag"].tile([P, RL, OWH], F32, tag="ostag")
    ob = pools["ostag"].tile([P, RL, OWH], F32, tag="ostag")
    ch = cs // 2
    nc.scalar.copy(ot[:, :, 0:ch], Od[:, :, 0:cs:2])
    nc.scalar.copy(ob[:, :, 0:ch], Od[:, :, 1:cs:2])
    nc.scalar.copy(ot[:, :, ch:OWH], Op[:, :, 0:OW - cs:2])
    nc.scalar.copy(ob[:, :, ch:OWH], Op[:, :, 1:OW - cs:2])
    yt = y[img, 0:HH, :].rearrange("(p i) w -> p i w", i=RL)
    yb = y[img, HH:H, :].rearrange("(p i) w -> p i w", i=RL)
    nc.sync.dma_start(yt[:, :, co:co + OWH], ot[:])
    nc.sync.dma_start(yb[:, :, co:co + OWH], ob[:])


def _emit_block(nc, pools, x, y, img, blk):
    """One (image, column-block) pass: 256 output columns x 512 rows."""
    tt = nc.vector.tensor_tensor
    c0 = 0 if blk == 0 else (W // 2 - 1) * C          # f32 col offset (0 / 765)
    co = 0 if blk == 0 else OWH                       # out f32 col offset

    # ---- DMA in: per half, rows (2p-1 .. 2p+2) clamped at image edges ----
    stag_t = pools["stag"].tile([P, 4, INW], F32, tag="stag")
    stag_b = pools["stag"].tile([P, 4, INW], F32, tag="stag")
    xi = x[img]                                        # [H, WC]
    ce = c0 + INW
    # main rows as ONE overlapping-AP DMA per half: slot j holds row 2p+j
    # (top) / HH-1+2p+j (bottom) for j=0..2
    vt = xi.copy()
    vt.ap = mybir.VecI64Pair([[2 * WC, P], [WC, 3], [1, INW]])
    vt.offset = xi.offset + c0
    nc.sync.dma_start(stag_t[:, 1:4, :], vt)
    vb = xi.copy()
    vb.ap = mybir.VecI64Pair([[2 * WC, P], [WC, 3], [1, INW]])
    vb.offset = xi.offset + (HH - 1) * WC + c0
    nc.sync.dma_start(stag_b[:, 0:3, :], vb)
    # top halo: row 2p-1 (p>=1); p=0 clamps to row 0
    nc.sync.dma_start(stag_t[1:P, 0, :], xi[1:2 * P - 2:2, c0:ce])
    nc.sync.dma_start(stag_t[0:1, 0, :], xi[0:1, c0:ce])
    # bottom halo: row 256+2p+2 (p<=126); p=127 clamps to row 511
    nc.sync.dma_start(stag_b[0:P - 1, 3, :], xi[HH + 2:H - 1:2, c0:ce])
    nc.sync.dma_start(stag_b[P - 1:P, 3, :], xi[H - 1:H, c0:ce])

    # ---- cast f32 -> fp16, interleaving the two halves ----
    X = pools["x"].tile([P, 4, XW], F16, tag="x")
    if blk == 0:
        # pixels 0..256 at stored px 1..257; stored px 0 = left pad
        nc.scalar.copy(X[:, :, 6:XW:2], stag_t[:, :, :])
        nc.scalar.copy(X[:, :, 7:XW:2], stag_b[:, :, :])
        nc.scalar.copy(X[:, :, 0:6], X[:, :, 6:12])            # replicate pad
    else:
        # pixels 255..511 at stored px 0..256; stored px 257 = right pad
        nc.scalar.copy(X[:, :, 0:LW:2], stag_t[:, :, :])
        nc.scalar.copy(X[:, :, 1:LW:2], stag_b[:, :, :])
        nc.scalar.copy(X[:, :, LW:XW], X[:, :, OW:LW])         # replicate pad

    # ---- vertical pass: column sort3 with shared pairwise min/max ----
    def wt():
        return pools["work"].tile([P, RL, XW], F16, tag="work", name="work")

    # output row i's window is X slots (i, i+1, i+2); only pairs (0,1) and
    # (1,2) are consumed, so compute exactly those two pair-rows.
    pmin = pools["pp"].tile([P, RL, XW], F16, tag="pp")
    pmax = pools["pp"].tile([P, RL, XW], F16, tag="pp")
    tt(pmin[:], X[:, 0:2, :], X[:, 1:3, :], op=MIN)
    tt(pmax[:], X[:, 0:2, :], X[:, 1:3, :], op=MAX)
    lo, tq, hi, mid = wt(), wt(), wt(), wt()
    tt(lo[:], pmin[:], X[:, 2:4, :], op=MIN)           # min3
    tt(tq[:], pmax[:], X[:, 2:4, :], op=MIN)
    tt(hi[:], pmax[:], X[:, 2:4, :], op=MAX)           # max3
    tt(mid[:], pmin[:], tq[:], op=MAX)                 # med3

    # ---- horizontal pass (all shifts are even element offsets) ----
    # pair results are only consumed at [0:OW] (as index k-1 of a window
    # centered at k), so compute exactly that span.
    # max3(lo)/min3(hi) use the even-odd pair factorization: a half-width op
    # builds the shared pair m[j] = op(px[2j+1], px[2j+2]); two half-width
    # ops fold it into even/odd outputs -> 1.5 elem-ops/output instead of 2.
    def pxg(t, w=XW):
        return t[:, :, 0:w].rearrange("p r (g c) -> p r g c", c=6)

    GH = OW // 12                                     # output pixel pairs
    loG, hiG = pxg(lo), pxg(hi)
    mA, mC = wt(), wt()
    mAG, mCG = pxg(mA, GH * 6), pxg(mC, GH * 6)
    tt(mAG[:], loG[:, :, 1:2 * GH + 1:2, :], loG[:, :, 2:2 * GH + 2:2, :], op=MAX)
    tt(mCG[:], hiG[:, :, 1:2 * GH + 1:2, :], hiG[:, :, 2:2 * GH + 2:2, :], op=MIN)
    A, Cm = wt(), wt()
    AG, CG = pxg(A, OW), pxg(Cm, OW)
    tt(AG[:, :, 0:2 * GH:2, :], loG[:, :, 0:2 * GH:2, :], mAG[:], op=MAX)
    tt(AG[:, :, 1:2 * GH:2, :], mAG[:], loG[:, :, 3:2 * GH + 2:2, :], op=MAX)
    tt(CG[:, :, 0:2 * GH:2, :], hiG[:, :, 0:2 * GH:2, :], mCG[:], op=MIN)
    tt(CG[:, :, 1:2 * GH:2, :], mCG[:], hiG[:, :, 3:2 * GH + 2:2, :], op=MIN)
    p_, q_ = wt(), wt()
    tt(p_[:, :, 0:OW], mid[:, :, 0:OW], mid[:, :, 6:6 + OW], op=MIN)
    tt(q_[:, :, 0:OW], mid[:, :, 0:OW], mid[:, :, 6:6 + OW], op=MAX)
    r_ = wt()
    tt(r_[:, :, 0:OW], q_[:, :, 0:OW], mid[:, :, 12:XW], op=MIN)
    Bm = wt()
    tt(Bm[:, :, 0:OW], p_[:, :, 0:OW], r_[:, :, 0:OW], op=MAX)    # med3(mid)
    s_, u_, v_ = wt(), wt(), wt()
    tt(s_[:, :, 0:OW], A[:, :, 0:OW], Bm[:, :, 0:OW], op=MIN)
    tt(u_[:, :, 0:OW], A[:, :, 0:OW], Bm[:, :, 0:OW], op=MAX)
    tt(v_[:, :, 0:OW], u_[:, :, 0:OW], Cm[:, :, 0:OW], op=MIN)
    O = pools["o"].tile([P, RL, OW], F16, tag="o")
    tt(O[:], s_[:, :, 0:OW], v_[:, :, 0:OW], op=MAX)              # med9

    # ---- de-interleave cast back to f32 and DMA out ----
    ot = pools["ostag"].tile([P, RL, OWH], F32, tag="ostag")
    ob = pools["ostag"].tile([P, RL, OWH], F32, tag="ostag")
    nc.scalar.copy(ot[:], O[:, :, 0:OW:2])
    nc.scalar.copy(ob[:], O[:, :, 1:OW:2])
    yt = y[img, 0:HH, :].rearrange("(p i) w -> p i w", i=RL)
    yb = y[img, HH:H, :].rearrange("(p i) w -> p i w", i=RL)
    nc.sync.dma_start(yt[:, :, co:co + OWH], ot[:])
    nc.sync.dma_start(yb[:, :, co:co + OWH], ob[:])


def _emit_block_rl4(nc, pools, x, y, img, blk):
    """One (image, column-block) pass, non-interleaved RL=4 layout.

    Partition p holds image rows 4p..4p+3; staging has 6 row-slots
    (4p-1..4p+4, edge-clamped) so each input row is read from HBM 1.5x
    instead of the interleaved layout's 2x. bf16 needs no interleave
    (no 16-bit 2x DVE mode exists on HW), so pixels stay contiguous:
    1-px shifts are 3-element offsets and the output casts/DMA are
    contiguous.
    """
    tt = nc.vector.tensor_tensor
    R4 = 4                                             # rows per partition
    OW3 = W // 2 * C                                   # 768 out elems/row
    XW3 = OW3 + 2 * C                                  # 774 with halo px
    c0 = 0 if blk == 0 else (W // 2 - 1) * C
    co = 0 if blk == 0 else OW3
    ce = c0 + INW

    # ---- DMA in: slots 1..4 = rows 4p..4p+3 (each row once) ----
    stag = pools["stag"].tile([P, 6, INW], F32, tag="stag")
    xi = x[img]
    vm = xi.copy()
    vm.ap = mybir.VecI64Pair([[R4 * WC, P], [WC, R4], [1, INW]])
    vm.offset = xi.offset + c0
    nc.sync.dma_start(stag[:, 1:5, :], vm)
    # slot 0 = row 4p-1 (p>=1); p=0 clamps to row 0
    nc.sync.dma_start(stag[1:P, 0, :], xi[3:H - 4:R4, c0:ce])
    nc.sync.dma_start(stag[0:1, 0, :], xi[0:1, c0:ce])
    # slot 5 = row 4p+4 (p<=126); p=127 clamps to row 511
    nc.sync.dma_start(stag[0:P - 1, 5, :], xi[R4:H - 3:R4, c0:ce])
    nc.sync.dma_start(stag[P - 1:P, 5, :], xi[H - 1:H, c0:ce])

    # ---- cast f32 -> bf16 (contiguous) + 1-px replicate pad ----
    X = pools["x"].tile([P, 6, XW3], F16, tag="x")
    if blk == 0:
        nc.scalar.copy(X[:, :, C:XW3], stag[:, :, :])
        nc.scalar.copy(X[:, :, 0:C], X[:, :, C:2 * C])
    else:
        nc.scalar.copy(X[:, :, 0:INW], stag[:, :, :])
        nc.scalar.copy(X[:, :, INW:XW3], X[:, :, INW - C:INW])

    # ---- vertical pass: sort3 of rows with shared pairwise min/max ----
    def wt():
        return pools["work"].tile([P, R4, XW3], F16, tag="work", name="w")

    pmin = pools["pp"].tile([P, 5, XW3], F16, tag="pp")
    pmax = pools["pp"].tile([P, 5, XW3], F16, tag="pp")
    tt(pmin[:], X[:, 0:5, :], X[:, 1:6, :], op=MIN)
    tt(pmax[:], X[:, 0:5, :], X[:, 1:6, :], op=MAX)
    lo, tq, hi, mid = wt(), wt(), wt(), wt()
    tt(lo[:], pmin[:, 0:R4, :], X[:, 2:6, :], op=MIN)
    tt(tq[:], pmax[:, 0:R4, :], X[:, 2:6, :], op=MIN)
    tt(hi[:], pmax[:, 0:R4, :], X[:, 2:6, :], op=MAX)
    tt(mid[:], pmin[:, 0:R4, :], tq[:], op=MAX)

    # ---- horizontal pass (1-px shift = 3 elems) ----
    def pxg(t, w=XW3):
        return t[:, :, 0:w].rearrange("p r (g c) -> p r g c", c=C)

    GH = OW3 // (2 * C)                                # output pixel pairs
    loG, hiG = pxg(lo), pxg(hi)
    mA, mC = wt(), wt()
    mAG, mCG = pxg(mA, GH * C), pxg(mC, GH * C)
    tt(mAG[:], loG[:, :, 1:2 * GH + 1:2, :], loG[:, :, 2:2 * GH + 2:2, :], op=MAX)
    tt(mCG[:], hiG[:, :, 1:2 * GH + 1:2, :], hiG[:, :, 2:2 * GH + 2:2, :], op=MIN)
    A, Cm = wt(), wt()
    AG, CG = pxg(A, OW3), pxg(Cm, OW3)
    tt(AG[:, :, 0:2 * GH:2, :], loG[:, :, 0:2 * GH:2, :], mAG[:], op=MAX)
    tt(AG[:, :, 1:2 * GH:2, :], mAG[:], loG[:, :, 3:2 * GH + 2:2, :], op=MAX)
    tt(CG[:, :, 0:2 * GH:2, :], hiG[:, :, 0:2 * GH:2, :], mCG[:], op=MIN)
    tt(CG[:, :, 1:2 * GH:2, :], mCG[:], hiG[:, :, 3:2 * GH + 2:2, :], op=MIN)
    p_, q_ = wt(), wt()
    tt(p_[:, :, 0:OW3], mid[:, :, 0:OW3], mid[:, :, C:C + OW3], op=MIN)
    tt(q_[:, :, 0:OW3], mid[:, :, 0:OW3], mid[:, :, C:C + OW3], op=MAX)
    r_ = wt()
    tt(r_[:, :, 0:OW3], q_[:, :, 0:OW3], mid[:, :, 2 * C:2 * C + OW3], op=MIN)
    Bm = wt()
    tt(Bm[:, :, 0:OW3], p_[:, :, 0:OW3], r_[:, :, 0:OW3], op=MAX)
    s_, u_, v_ = wt(), wt(), wt()
    tt(s_[:, :, 0:OW3], A[:, :, 0:OW3], Bm[:, :, 0:OW3], op=MIN)
    tt(u_[:, :, 0:OW3], A[:, :, 0:OW3], Bm[:, :, 0:OW3], op=MAX)
    tt(v_[:, :, 0:OW3], u_[:, :, 0:OW3], Cm[:, :, 0:OW3], op=MIN)
    O = pools["o"].tile([P, R4, OW3], F16, tag="o")
    tt(O[:], s_[:, :, 0:OW3], v_[:, :, 0:OW3], op=MAX)            # med9

    # ---- cast back to f32 (contiguous) and one DMA out ----
    ot = pools["ostag"].tile([P, R4, OW3], F32, tag="ostag")
    nc.scalar.copy(ot[:], O[:])
    yv = y[img].rearrange("(p j) w -> p j w", j=R4)
    nc.sync.dma_start(yv[:, :, co:co + OW3], ot[:])


def _emit_block_rl4c(nc, pools, x, y, img, blk):
    """One (image, column-block) pass, RL=4 layout, 15.0 ops/elem network.

    Improvements over _emit_block_rl4 (17.55 ops/elem):
      * Vertical sort3 by inserting the outer row into a shared sorted pair:
        pairs (4p,4p+1),(4p+2,4p+3) are sorted once (pmin/pmax); each output
        row r folds its third ("outer") row in via
          lo3 = min(outer, pmin), hi3 = max(outer, pmax),
          mid3 = min(max(outer, pmin), pmax)
        -> 5.0 elem-ops per output instead of 6.5. Output rows land in
        permuted order [0,2,1,3] (even rows then odd rows of the quad) so
        both fold groups are single strided instructions; the output cast
        un-permutes for free with two strided Act copies.
      * Horizontal med3(mid) via the same shared-pair identity
        med3(a,b,c) = min(max(a, min(b,c)), max(b,c)) on even-odd pixel
        pairs -> 3.0 elem-ops instead of 4.0.
    """
    tt = nc.vector.tensor_tensor
    R4 = 4
    OW3 = W // 2 * C                                   # 768 out elems/row
    XW3 = OW3 + 2 * C                                  # 774 with halo px
    GH = OW3 // (2 * C)                                # 128 output pixel pairs
    c0 = 0 if blk == 0 else (W // 2 - 1) * C
    co = 0 if blk == 0 else OW3
    ce = c0 + INW

    # ---- DMA in: slots 1..4 = rows 4p..4p+3 (each row once) ----
    # Queue split (measured): the kernel is DMA-QUEUE-bound, not compute
    # bound. One HWDGE queue serving all DMAs caps at ~95 GB/s; spreading
    # main-in (SP) / halos (Pool SWDGE) / out (Act) runs at ~3.2x that.
    stag = pools["stag"].tile([P, 6, INW], F32, tag="stag")
    xi = x[img]
    vm = xi.copy()
    vm.ap = mybir.VecI64Pair([[R4 * WC, P], [WC, R4], [1, INW]])
    vm.offset = xi.offset + c0
    nc.sync.dma_start(stag[:, 1:5, :], vm)
    nc.gpsimd.dma_start(stag[1:P, 0, :], xi[3:H - 4:R4, c0:ce])
    nc.gpsimd.dma_start(stag[0:1, 0, :], xi[0:1, c0:ce])
    nc.gpsimd.dma_start(stag[0:P - 1, 5, :], xi[R4:H - 3:R4, c0:ce])
    nc.gpsimd.dma_start(stag[P - 1:P, 5, :], xi[H - 1:H, c0:ce])

    # ---- cast f32 -> bf16 (contiguous) + 1-px replicate pad ----
    X = pools["x"].tile([P, 6, XW3], F16, tag="x")
    if blk == 0:
        nc.scalar.copy(X[:, :, C:XW3], stag[:, :, :])
        nc.scalar.copy(X[:, :, 0:C], X[:, :, C:2 * C])
    else:
        nc.scalar.copy(X[:, :, 0:INW], stag[:, :, :])
        nc.scalar.copy(X[:, :, INW:XW3], X[:, :, INW - C:INW])

    def wt():
        return pools["work"].tile([P, R4, XW3], F16, tag="work", name="w")

    # ---- vertical: sorted pairs (slots 1,2) and (3,4), insert outer ----
    pmin = pools["pp"].tile([P, 2, XW3], F16, tag="pp")
    pmax = pools["pp"].tile([P, 2, XW3], F16, tag="pp")
    tt(pmin[:], X[:, 1:5:2, :], X[:, 2:6:2, :], op=MIN)
    tt(pmax[:], X[:, 1:5:2, :], X[:, 2:6:2, :], op=MAX)
    # output row order [4p, 4p+2, 4p+1, 4p+3]: evens use outer slots {0,2},
    # odds use outer slots {3,5}; both against pair rows {0,1}.
    lo, hi, t_, mid = wt(), wt(), wt(), wt()
    tt(lo[:, 0:2, :], X[:, 0:3:2, :], pmin[:], op=MIN)
    tt(lo[:, 2:4, :], X[:, 3:6:2, :], pmin[:], op=MIN)
    tt(hi[:, 0:2, :], X[:, 0:3:2, :], pmax[:], op=MAX)
    tt(hi[:, 2:4, :], X[:, 3:6:2, :], pmax[:], op=MAX)
    tt(t_[:, 0:2, :], X[:, 0:3:2, :], pmin[:], op=MAX)
    tt(t_[:, 2:4, :], X[:, 3:6:2, :], pmin[:], op=MAX)
    tt(mid[:, 0:2, :], t_[:, 0:2, :], pmax[:], op=MIN)
    tt(mid[:, 2:4, :], t_[:, 2:4, :], pmax[:], op=MIN)

    # ---- horizontal (1-px shift = 3 elems; even-odd pixel pairs) ----
    def pxg(t, w=XW3):
        return t[:, :, 0:w].rearrange("p r (g c) -> p r g c", c=C)

    loG, hiG, midG = pxg(lo), pxg(hi), pxg(mid)
    mA, mC, pm, pM = wt(), wt(), wt(), wt()
    mAG, mCG = pxg(mA, GH * C), pxg(mC, GH * C)
    pmG, pMG = pxg(pm, GH * C), pxg(pM, GH * C)
    tt(mAG[:], loG[:, :, 1:2 * GH + 1:2, :], loG[:, :, 2:2 * GH + 2:2, :], op=MAX)
    tt(mCG[:], hiG[:, :, 1:2 * GH + 1:2, :], hiG[:, :, 2:2 * GH + 2:2, :], op=MIN)
    tt(pmG[:], midG[:, :, 1:2 * GH + 1:2, :], midG[:, :, 2:2 * GH + 2:2, :], op=MIN)
    tt(pMG[:], midG[:, :, 1:2 * GH + 1:2, :], midG[:, :, 2:2 * GH + 2:2, :], op=MAX)
    A, Cm, tb, Bm = wt(), wt(), wt(), wt()
    AG, CG = pxg(A, OW3), pxg(Cm, OW3)
    tbG, BG = pxg(tb, OW3), pxg(Bm, OW3)
    tt(AG[:, :, 0:2 * GH:2, :], loG[:, :, 0:2 * GH:2, :], mAG[:], op=MAX)
    tt(AG[:, :, 1:2 * GH:2, :], mAG[:], loG[:, :, 3:2 * GH + 2:2, :], op=MAX)
    tt(CG[:, :, 0:2 * GH:2, :], hiG[:, :, 0:2 * GH:2, :], mCG[:], op=MIN)
    tt(CG[:, :, 1:2 * GH:2, :], mCG[:], hiG[:, :, 3:2 * GH + 2:2, :], op=MIN)
    # med3(mid) = min(max(outer, pm), pM)
    tt(tbG[:, :, 0:2 * GH:2, :], midG[:, :, 0:2 * GH:2, :], pmG[:], op=MAX)
    tt(tbG[:, :, 1:2 * GH:2, :], pmG[:], midG[:, :, 3:2 * GH + 2:2, :], op=MAX)
    tt(BG[:, :, 0:2 * GH:2, :], tbG[:, :, 0:2 * GH:2, :], pMG[:], op=MIN)
    tt(BG[:, :, 1:2 * GH:2, :], tbG[:, :, 1:2 * GH:2, :], pMG[:], op=MIN)
    # ---- final med3(A, Bm, Cm) ----
    s_, u_, v_ = wt(), wt(), wt()
    tt(s_[:, :, 0:OW3], A[:, :, 0:OW3], Bm[:, :, 0:OW3], op=MIN)
    tt(u_[:, :, 0:OW3], A[:, :, 0:OW3], Bm[:, :, 0:OW3], op=MAX)
    tt(v_[:, :, 0:OW3], u_[:, :, 0:OW3], Cm[:, :, 0:OW3], op=MIN)
    O = pools["o"].tile([P, R4, OW3], F16, tag="o")
    tt(O[:], s_[:, :, 0:OW3], v_[:, :, 0:OW3], op=MAX)            # med9

    # ---- cast back to f32, un-permuting rows [0,2,1,3] -> [0,1,2,3] ----
    ot = pools["ostag"].tile([P, R4, OW3], F32, tag="ostag")
    nc.scalar.copy(ot[:, 0:3:2, :], O[:, 0:2, :])
    nc.scalar.copy(ot[:, 1:4:2, :], O[:, 2:4, :])
    yv = y[img].rearrange("(p j) w -> p j w", j=R4)
    nc.scalar.dma_start(yv[:, :, co:co + OW3], ot[:])


def _emit_block_rl4d(nc, pools, x, y, img, blk, dve_reps=1):
    """One (image, column-block) pass optimized for DVE instruction cost.

    Measured reality: per-instruction cost is dominated by AP shape, not
    element count (contiguous [4,768] ~ 0.4 ns/elem; 4D grouped-pixel APs
    ~0.8 ns/elem + overhead). So: old-style contiguous vertical pass, and a
    plain 3-tap horizontal pass (19.1 elem-ops/output but only 18
    contiguous DVE instructions, no strided slots, no 4D APs).
    """
    tt = nc.vector.tensor_tensor
    R4 = 4
    OW3 = W // 2 * C                                   # 768 out elems/row
    XW3 = OW3 + 2 * C                                  # 774 with halo px
    c0 = 0 if blk == 0 else (W // 2 - 1) * C
    co = 0 if blk == 0 else OW3
    ce = c0 + INW

    # ---- DMA in (queue-split: main on SP, halos on Pool SWDGE) ----
    stag = pools["stag"].tile([P, 6, INW], F32, tag="stag")
    xi = x[img]
    vm = xi.copy()
    vm.ap = mybir.VecI64Pair([[R4 * WC, P], [WC, R4], [1, INW]])
    vm.offset = xi.offset + c0
    nc.sync.dma_start(stag[:, 1:5, :], vm)
    nc.gpsimd.dma_start(stag[1:P, 0, :], xi[3:H - 4:R4, c0:ce])
    nc.gpsimd.dma_start(stag[0:1, 0, :], xi[0:1, c0:ce])
    nc.gpsimd.dma_start(stag[0:P - 1, 5, :], xi[R4:H - 3:R4, c0:ce])
    nc.gpsimd.dma_start(stag[P - 1:P, 5, :], xi[H - 1:H, c0:ce])

    # ---- cast f32 -> bf16 (contiguous) + 1-px replicate pad ----
    X = pools["x"].tile([P, 6, XW3], F16, tag="x")
    if blk == 0:
        nc.scalar.copy(X[:, :, C:XW3], stag[:, :, :])
        nc.scalar.copy(X[:, :, 0:C], X[:, :, C:2 * C])
    else:
        nc.scalar.copy(X[:, :, 0:INW], stag[:, :, :])
        nc.scalar.copy(X[:, :, INW:XW3], X[:, :, INW - C:INW])

    def wt():
        return pools["work"].tile([P, R4, XW3], F16, tag="work", name="w")

    def med9_chain():
        # ---- vertical sort3, shared pairwise min/max, all contiguous ----
        pmin = pools["pp"].tile([P, 5, XW3], F16, tag="pp")
        pmax = pools["pp"].tile([P, 5, XW3], F16, tag="pp")
        tt(pmin[:], X[:, 0:5, :], X[:, 1:6, :], op=MIN)
        tt(pmax[:], X[:, 0:5, :], X[:, 1:6, :], op=MAX)
        lo, tq, hi, mid = wt(), wt(), wt(), wt()
        tt(lo[:], pmin[:, 0:R4, :], X[:, 2:6, :], op=MIN)
        tt(tq[:], pmax[:, 0:R4, :], X[:, 2:6, :], op=MIN)
        tt(hi[:], pmax[:, 0:R4, :], X[:, 2:6, :], op=MAX)
        tt(mid[:], pmin[:, 0:R4, :], tq[:], op=MAX)

        # ---- horizontal: plain 3-tap max3/min3/med3, contiguous [4,768] ----
        t1, A, c1, Cm = wt(), wt(), wt(), wt()
        tt(t1[:, :, 0:OW3], lo[:, :, 0:OW3], lo[:, :, C:C + OW3], op=MAX)
        tt(A[:, :, 0:OW3], t1[:, :, 0:OW3], lo[:, :, 2 * C:XW3], op=MAX)
        tt(c1[:, :, 0:OW3], hi[:, :, 0:OW3], hi[:, :, C:C + OW3], op=MIN)
        tt(Cm[:, :, 0:OW3], c1[:, :, 0:OW3], hi[:, :, 2 * C:XW3], op=MIN)
        p_, q_, r_, Bm = wt(), wt(), wt(), wt()
        tt(p_[:, :, 0:OW3], mid[:, :, 0:OW3], mid[:, :, C:C + OW3], op=MIN)
        tt(q_[:, :, 0:OW3], mid[:, :, 0:OW3], mid[:, :, C:C + OW3], op=MAX)
        tt(r_[:, :, 0:OW3], q_[:, :, 0:OW3], mid[:, :, 2 * C:XW3], op=MIN)
        tt(Bm[:, :, 0:OW3], p_[:, :, 0:OW3], r_[:, :, 0:OW3], op=MAX)
        s_, u_, v_ = wt(), wt(), wt()
        tt(s_[:, :, 0:OW3], A[:, :, 0:OW3], Bm[:, :, 0:OW3], op=MIN)
        tt(u_[:, :, 0:OW3], A[:, :, 0:OW3], Bm[:, :, 0:OW3], op=MAX)
        tt(v_[:, :, 0:OW3], u_[:, :, 0:OW3], Cm[:, :, 0:OW3], op=MIN)
        O = pools["o"].tile([P, R4, OW3], F16, tag="o")
        tt(O[:], s_[:, :, 0:OW3], v_[:, :, 0:OW3], op=MAX)        # med9
        return O

    if dve_reps == 0:
        O = pools["o"].tile([P, R4, OW3], F16, tag="o")
        tt(O[:], X[:, 0:4, 0:OW3], X[:, 1:5, 0:OW3], op=MIN)      # placeholder
    else:
        O = med9_chain()
        for _ in range(dve_reps - 1):
            O2 = med9_chain()
            O3 = pools["o"].tile([P, R4, OW3], F16, tag="o")
            tt(O3[:], O[:], O2[:], op=MIN)                        # == median
            O = O3

    # ---- cast back to f32 (contiguous) and one DMA out on Act queue ----
    ot = pools["ostag"].tile([P, R4, OW3], F32, tag="ostag")
    nc.scalar.copy(ot[:], O[:])
    yv = y[img].rearrange("(p j) w -> p j w", j=R4)
    nc.scalar.dma_start(yv[:, :, co:co + OW3], ot[:])


def _emit_block_rl4e(nc, pools, x, y, img, blk):
    """Best measured combination: rl4c's strided-slot vertical (insert into
    sorted pair, ~0.29 ns/elem) + rl4d's contiguous 3-tap horizontal
    (~0.41 ns/elem), DMA spread over all three queues with the 1-descriptor
    clamp DMAs kept off the Q7 SWDGE path (~1us fixed cost each there).
    """
    tt = nc.vector.tensor_tensor
    R4 = 4
    OW3 = W // 2 * C                                   # 768 out elems/row
    XW3 = OW3 + 2 * C                                  # 774 with halo px
    c0 = 0 if blk == 0 else (W // 2 - 1) * C
    co = 0 if blk == 0 else OW3
    ce = c0 + INW

    # ---- DMA in: SP main rows (f32 staging); halos as CASTING SWDGE DMAs
    # straight into X (bf16) on the Pool queue.
    stag = pools["stag"].tile([P, 4, INW], F32, tag="stag")
    X = pools["x"].tile([P, 6, XW3], F16, tag="x")
    xo = C if blk == 0 else 0                          # X col offset of px 0
    xi = x[img]
    vm = xi.copy()
    vm.ap = mybir.VecI64Pair([[R4 * WC, P], [WC, R4], [1, INW]])
    vm.offset = xi.offset + c0
    nc.sync.dma_start(stag[:], vm)
    nc.gpsimd.dma_start(X[1:P, 0, xo:xo + INW], xi[3:H - 4:R4, c0:ce])
    nc.gpsimd.dma_start(X[0:1, 0, xo:xo + INW], xi[0:1, c0:ce])
    nc.gpsimd.dma_start(X[0:P - 1, 5, xo:xo + INW], xi[R4:H - 3:R4, c0:ce])
    nc.gpsimd.dma_start(X[P - 1:P, 5, xo:xo + INW], xi[H - 1:H, c0:ce])

    # ---- cast main rows f32 -> bf16 + 1-px replicate pad ----
    if blk == 0:
        nc.scalar.copy(X[:, 1:5, C:XW3], stag[:])
        nc.scalar.copy(X[:, :, 0:C], X[:, :, C:2 * C])
    else:
        nc.scalar.copy(X[:, 1:5, 0:INW], stag[:])
        nc.scalar.copy(X[:, :, INW:XW3], X[:, :, INW - C:INW])

    def wt():
        return pools["work"].tile([P, R4, XW3], F16, tag="work", name="w")

    # ---- vertical: sorted pairs (slots 1,2),(3,4); insert outer row ----
    # output row order [4p, 4p+2, 4p+1, 4p+3] (evens then odds of the quad)
    pmin = pools["pp"].tile([P, 2, XW3], F16, tag="pp")
    pmax = pools["pp"].tile([P, 2, XW3], F16, tag="pp")
    tt(pmin[:], X[:, 1:5:2, :], X[:, 2:6:2, :], op=MIN)
    tt(pmax[:], X[:, 1:5:2, :], X[:, 2:6:2, :], op=MAX)
    lo, hi, t_, mid = wt(), wt(), wt(), wt()
    tt(lo[:, 0:2, :], X[:, 0:3:2, :], pmin[:], op=MIN)
    tt(lo[:, 2:4, :], X[:, 3:6:2, :], pmin[:], op=MIN)
    tt(hi[:, 0:2, :], X[:, 0:3:2, :], pmax[:], op=MAX)
    tt(hi[:, 2:4, :], X[:, 3:6:2, :], pmax[:], op=MAX)
    tt(t_[:, 0:2, :], X[:, 0:3:2, :], pmin[:], op=MAX)
    tt(t_[:, 2:4, :], X[:, 3:6:2, :], pmin[:], op=MAX)
    tt(mid[:, 0:2, :], t_[:, 0:2, :], pmax[:], op=MIN)
    tt(mid[:, 2:4, :], t_[:, 2:4, :], pmax[:], op=MIN)

    # ---- horizontal: plain 3-tap max3/min3/med3, contiguous [4,768] ----
    t1, A, c1, Cm = wt(), wt(), wt(), wt()
    tt(t1[:, :, 0:OW3], lo[:, :, 0:OW3], lo[:, :, C:C + OW3], op=MAX)
    tt(A[:, :, 0:OW3], t1[:, :, 0:OW3], lo[:, :, 2 * C:XW3], op=MAX)
    tt(c1[:, :, 0:OW3], hi[:, :, 0:OW3], hi[:, :, C:C + OW3], op=MIN)
    tt(Cm[:, :, 0:OW3], c1[:, :, 0:OW3], hi[:, :, 2 * C:XW3], op=MIN)
    p_, q_, r_, Bm = wt(), wt(), wt(), wt()
    tt(p_[:, :, 0:OW3], mid[:, :, 0:OW3], mid[:, :, C:C + OW3], op=MIN)
    tt(q_[:, :, 0:OW3], mid[:, :, 0:OW3], mid[:, :, C:C + OW3], op=MAX)
    tt(r_[:, :, 0:OW3], q_[:, :, 0:OW3], mid[:, :, 2 * C:XW3], op=MIN)
    tt(Bm[:, :, 0:OW3], p_[:, :, 0:OW3], r_[:, :, 0:OW3], op=MAX)
    s_, u_, v_ = wt(), wt(), wt()
    tt(s_[:, :, 0:OW3], A[:, :, 0:OW3], Bm[:, :, 0:OW3], op=MIN)
    tt(u_[:, :, 0:OW3], A[:, :, 0:OW3], Bm[:, :, 0:OW3], op=MAX)
    tt(v_[:, :, 0:OW3], u_[:, :, 0:OW3], Cm[:, :, 0:OW3], op=MIN)
    O = pools["o"].tile([P, R4, OW3], F16, tag="o")
    tt(O[:], s_[:, :, 0:OW3], v_[:, :, 0:OW3], op=MAX)            # med9

    # ---- cast back to f32 un-permuting rows; DMA out on Act queue ----
    ot = pools["ostag"].tile([P, R4, OW3], F32, tag="ostag")
    nc.scalar.copy(ot[:, 0:3:2, :], O[:, 0:2, :])
    nc.scalar.copy(ot[:, 1:4:2, :], O[:, 2:4, :])
    yv = y[img].rearrange("(p j) w -> p j w", j=R4)
    nc.scalar.dma_start(yv[:, :, co:co + OW3], ot[:])


def _emit_image_zip(nc, pools, x, y, img):
    """One whole image, the two column-block med9 chains interleaved.

    Consecutive DVE instructions alternate between the independent L/R
    block chains, hiding RAW latency (measured ~14% on back-to-back
    independent ops) and letting each block's loads/casts overlap the
    other block's compute.
    """
    tt = nc.vector.tensor_tensor
    R4 = 4
    OW3 = W // 2 * C
    XW3 = OW3 + 2 * C

    def load(blk):
        c0 = 0 if blk == 0 else (W // 2 - 1) * C
        ce = c0 + INW
        stag = pools["stag"].tile([P, 4, INW], F32, tag="stag")
        X = pools["x"].tile([P, 6, XW3], F16, tag="x")
        xo = C if blk == 0 else 0
        xi = x[img]
        vm = xi.copy()
        vm.ap = mybir.VecI64Pair([[R4 * WC, P], [WC, R4], [1, INW]])
        vm.offset = xi.offset + c0
        nc.sync.dma_start(stag[:], vm)
        nc.gpsimd.dma_start(X[1:P, 0, xo:xo + INW], xi[3:H - 4:R4, c0:ce])
        nc.gpsimd.dma_start(X[0:1, 0, xo:xo + INW], xi[0:1, c0:ce])
        nc.gpsimd.dma_start(X[0:P - 1, 5, xo:xo + INW], xi[R4:H - 3:R4, c0:ce])
        nc.gpsimd.dma_start(X[P - 1:P, 5, xo:xo + INW], xi[H - 1:H, c0:ce])
        if blk == 0:
            nc.scalar.copy(X[:, 1:5, C:XW3], stag[:])
            nc.scalar.copy(X[:, :, 0:C], X[:, :, C:2 * C])
        else:
            nc.scalar.copy(X[:, 1:5, 0:INW], stag[:])
            nc.scalar.copy(X[:, :, INW:XW3], X[:, :, INW - C:INW])
        return X

    def steps(X, blk):
        """DVE chain as a list of closures (one instruction each)."""
        co = 0 if blk == 0 else OW3
        st = {}

        def wt():
            return pools["work"].tile([P, R4, XW3], F16, tag="work", name="w")

        def s_vert():
            st["pmin"] = pools["pp"].tile([P, 2, XW3], F16, tag="pp", name="pmin")
            st["pmax"] = pools["pp"].tile([P, 2, XW3], F16, tag="pp", name="pmax")
            tt(st["pmin"][:], X[:, 1:5:2, :], X[:, 2:6:2, :], op=MIN)

        def fin():
            ot = pools["ostag"].tile([P, R4, OW3], F32, tag="ostag")
            nc.scalar.copy(ot[:, 0:3:2, :], st["O"][:, 0:2, :])
            nc.scalar.copy(ot[:, 1:4:2, :], st["O"][:, 2:4, :])
            yv = y[img].rearrange("(p j) w -> p j w", j=R4)
            nc.scalar.dma_start(yv[:, :, co:co + OW3], ot[:])

        seq = [s_vert,
               lambda: tt(st["pmax"][:], X[:, 1:5:2, :], X[:, 2:6:2, :], op=MAX)]
        for nm, args in [
            ("lo", (0, MIN, "pmin")), ("hi", (0, MAX, "pmax")),
            ("t_", (0, MAX, "pmin"))]:
            def fold(nm=nm, op=args[1], pair=args[2]):
                t = wt()
                st[nm] = t
                tt(t[:, 0:2, :], X[:, 0:3:2, :], st[pair][:], op=op)
                tt(t[:, 2:4, :], X[:, 3:6:2, :], st[pair][:], op=op)
            seq.append(fold)

        def mid():
            t = wt()
            st["mid"] = t
            tt(t[:, 0:2, :], st["t_"][:, 0:2, :], st["pmax"][:], op=MIN)
            tt(t[:, 2:4, :], st["t_"][:, 2:4, :], st["pmax"][:], op=MIN)
        seq.append(mid)

        def h(nm, a, ash, b, bsh, op):
            def go():
                t = wt()
                st[nm] = t
                src_a = st[a][:, :, ash:ash + OW3] if ash is not None else st[a][:, :, 0:OW3]
                src_b = st[b][:, :, bsh:bsh + OW3] if bsh is not None else st[b][:, :, 0:OW3]
                tt(t[:, :, 0:OW3], src_a, src_b, op=op)
            return go

        seq += [
            h("t1", "lo", 0, "lo", C, MAX),
            h("A", "t1", 0, "lo", 2 * C, MAX),
            h("c1", "hi", 0, "hi", C, MIN),
            h("Cm", "c1", 0, "hi", 2 * C, MIN),
            h("p_", "mid", 0, "mid", C, MIN),
            h("q_", "mid", 0, "mid", C, MAX),
            h("r_", "q_", 0, "mid", 2 * C, MIN),
            h("Bm", "p_", 0, "r_", 0, MAX),
            h("s_", "A", 0, "Bm", 0, MIN),
            h("u_", "A", 0, "Bm", 0, MAX),
            h("v_", "u_", 0, "Cm", 0, MIN),
        ]

        def last():
            O = pools["o"].tile([P, R4, OW3], F16, tag="o", name="O")
            st["O"] = O
            tt(O[:], st["s_"][:, :, 0:OW3], st["v_"][:, :, 0:OW3], op=MAX)
        seq.append(last)
        seq.append(fin)
        return seq

    XL = load(0)
    XR = load(1)
    sl, sr = steps(XL, 0), steps(XR, 1)
    for a, b in zip(sl, sr):
        a()
        b()


def _rl4e_load(nc, pools, x, img, blk):
    """Load+cast phase of an rl4e block: DMAs on SP/Pool queues + Act cast.

    Emitted AHEAD of earlier blocks' compute tails (software pipelining) so
    the in-order Act engine never has a future block's input cast queued
    behind an out-cast that waits on the DVE chain.
    """
    R4 = 4
    OW3 = W // 2 * C
    XW3 = OW3 + 2 * C
    c0 = 0 if blk == 0 else (W // 2 - 1) * C
    ce = c0 + INW
    stag = pools["stag"].tile([P, 4, INW], F32, tag="stag")
    X = pools["x"].tile([P, 6, XW3], F16, tag="x")
    xo = C if blk == 0 else 0
    xi = x[img]
    vm = xi.copy()
    vm.ap = mybir.VecI64Pair([[R4 * WC, P], [WC, R4], [1, INW]])
    vm.offset = xi.offset + c0
    nc.sync.dma_start(stag[:], vm)
    nc.gpsimd.dma_start(X[1:P, 0, xo:xo + INW], xi[3:H - 4:R4, c0:ce])
    nc.gpsimd.dma_start(X[0:P - 1, 5, xo:xo + INW], xi[R4:H - 3:R4, c0:ce])
    nc.gpsimd.dma_start(X[P - 1:P, 5, xo:xo + INW], xi[H - 1:H, c0:ce])
    # top clamp as a tiny Act copy (row 0 == slot1 of p=0, already cast);
    # Act accesses may not start at partition 127, so the bottom clamp
    # stays a 1-descriptor SWDGE DMA.
    if blk == 0:
        nc.scalar.copy(X[:, 1:5, C:XW3], stag[:])
        nc.scalar.copy(X[0:1, 0, C:XW3], X[0:1, 1, C:XW3])
        nc.scalar.copy(X[:, :, 0:C], X[:, :, C:2 * C])
    else:
        nc.scalar.copy(X[:, 1:5, 0:INW], stag[:])
        nc.scalar.copy(X[0:1, 0, 0:INW], X[0:1, 1, 0:INW])
        nc.scalar.copy(X[:, :, INW:XW3], X[:, :, INW - C:INW])
    return X


def _rl4e_compute(nc, pools, y, img, blk, X):
    """DVE chain + out-cast + out-DMA of an rl4e block."""
    tt = nc.vector.tensor_tensor
    R4 = 4
    OW3 = W // 2 * C
    XW3 = OW3 + 2 * C
    co = 0 if blk == 0 else OW3

    def wt():
        return pools["work"].tile([P, R4, XW3], F16, tag="work", name="w")

    pmin = pools["pp"].tile([P, 2, XW3], F16, tag="pp")
    pmax = pools["pp"].tile([P, 2, XW3], F16, tag="pp")
    tt(pmin[:], X[:, 1:5:2, :], X[:, 2:6:2, :], op=MIN)
    tt(pmax[:], X[:, 1:5:2, :], X[:, 2:6:2, :], op=MAX)
    # Folds as single 4-row instructions: outer rows {0,2,3,5} via nested
    # slicing; the sorted-pair operand broadcast to rows [0,1,0,1] with a
    # stride-0 middle dim (verified exact on HW).
    Xo = X[:, 0:6, :].rearrange("p (a b) w -> p a b w", a=2)[:, :, 0:3:2, :]

    def bcast(pt):
        v = pt[:, 0:2, :].copy()
        v.ap = mybir.VecI64Pair([[2 * XW3, P], [0, 2], [XW3, 2], [1, XW3]])
        return v

    pminB, pmaxB = bcast(pmin), bcast(pmax)
    lo, hi, t_, mid = wt(), wt(), wt(), wt()

    def r4(tl):
        return tl[:].rearrange("p (a b) w -> p a b w", a=2)

    tt(r4(lo), Xo, pminB, op=MIN)
    tt(r4(hi), Xo, pmaxB, op=MAX)
    tt(r4(t_), Xo, pminB, op=MAX)
    tt(r4(mid), r4(t_), pmaxB, op=MIN)

    t1, A, c1, Cm = wt(), wt(), wt(), wt()
    tt(t1[:, :, 0:OW3], lo[:, :, 0:OW3], lo[:, :, C:C + OW3], op=MAX)
    tt(A[:, :, 0:OW3], t1[:, :, 0:OW3], lo[:, :, 2 * C:XW3], op=MAX)
    tt(c1[:, :, 0:OW3], hi[:, :, 0:OW3], hi[:, :, C:C + OW3], op=MIN)
    tt(Cm[:, :, 0:OW3], c1[:, :, 0:OW3], hi[:, :, 2 * C:XW3], op=MIN)
    p_, q_, r_, Bm = wt(), wt(), wt(), wt()
    tt(p_[:, :, 0:OW3], mid[:, :, 0:OW3], mid[:, :, C:C + OW3], op=MIN)
    tt(q_[:, :, 0:OW3], mid[:, :, 0:OW3], mid[:, :, C:C + OW3], op=MAX)
    tt(r_[:, :, 0:OW3], q_[:, :, 0:OW3], mid[:, :, 2 * C:XW3], op=MIN)
    tt(Bm[:, :, 0:OW3], p_[:, :, 0:OW3], r_[:, :, 0:OW3], op=MAX)
    s_, u_, v_ = wt(), wt(), wt()
    tt(s_[:, :, 0:OW3], A[:, :, 0:OW3], Bm[:, :, 0:OW3], op=MIN)
    tt(u_[:, :, 0:OW3], A[:, :, 0:OW3], Bm[:, :, 0:OW3], op=MAX)
    tt(v_[:, :, 0:OW3], u_[:, :, 0:OW3], Cm[:, :, 0:OW3], op=MIN)
    O = pools["o"].tile([P, R4, OW3], F16, tag="o")
    tt(O[:], s_[:, :, 0:OW3], v_[:, :, 0:OW3], op=MAX)

    ot = pools["ostag"].tile([P, R4, OW3], F32, tag="ostag")
    nc.scalar.copy(ot[:, 0:3:2, :], O[:, 0:2, :])
    nc.scalar.copy(ot[:, 1:4:2, :], O[:, 2:4, :])
    yv = y[img].rearrange("(p j) w -> p j w", j=R4)
    nc.scalar.dma_start(yv[:, :, co:co + OW3], ot[:])


def _fw_load(nc, pools, x, img):
    """Full-width image load: all input via casting SWDGE DMAs into X."""
    XWF = WC + 2 * C                                   # 1542
    X = pools["x"].tile([P, 6, XWF], F16, tag="x")
    xi = x[img]
    vm = xi.copy()
    vm.ap = mybir.VecI64Pair([[4 * WC, P], [WC, 4], [1, WC]])
    vm.offset = xi.offset
    nc.gpsimd.dma_start(X[:, 1:5, C:C + WC], vm)
    nc.gpsimd.dma_start(X[1:P, 0, C:C + WC], xi[3:H - 4:4, :])
    nc.gpsimd.dma_start(X[0:P - 1, 5, C:C + WC], xi[4:H - 3:4, :])
    nc.gpsimd.dma_start(X[P - 1:P, 5, C:C + WC], xi[H - 1:H, :])
    nc.scalar.copy(X[0:1, 0, C:C + WC], X[0:1, 1, C:C + WC])   # top clamp
    nc.scalar.copy(X[:, :, 0:C], X[:, :, C:2 * C])             # left pad
    nc.scalar.copy(X[:, :, C + WC:XWF], X[:, :, WC:C + WC])    # right pad
    return X


def _fw_compute(nc, pools, y, img, X):
    """Full-width med9 chain + output; one image per pass."""
    tt = nc.vector.tensor_tensor
    OWF = WC                                           # 1536
    XWF = WC + 2 * C                                   # 1542

    def wt():
        return pools["work"].tile([P, 4, XWF], F16, tag="work", name="w")

    pmin = pools["pp"].tile([P, 2, XWF], F16, tag="pp", name="pmin")
    pmax = pools["pp"].tile([P, 2, XWF], F16, tag="pp", name="pmax")
    tt(pmin[:], X[:, 1:5:2, :], X[:, 2:6:2, :], op=MIN)
    tt(pmax[:], X[:, 1:5:2, :], X[:, 2:6:2, :], op=MAX)
    Xo = X[:, 0:6, :].rearrange("p (a b) w -> p a b w", a=2)[:, :, 0:3:2, :]

    def bcast(pt):
        v = pt[:, 0:2, :].copy()
        v.ap = mybir.VecI64Pair([[2 * XWF, P], [0, 2], [XWF, 2], [1, XWF]])
        return v

    pminB, pmaxB = bcast(pmin), bcast(pmax)
    lo, hi, t_, mid = wt(), wt(), wt(), wt()

    def r4(tl):
        return tl[:].rearrange("p (a b) w -> p a b w", a=2)

    tt(r4(lo), Xo, pminB, op=MIN)
    tt(r4(hi), Xo, pmaxB, op=MAX)
    tt(r4(t_), Xo, pminB, op=MAX)
    tt(r4(mid), r4(t_), pmaxB, op=MIN)

    t1, A, c1, Cm = wt(), wt(), wt(), wt()
    tt(t1[:, :, 0:OWF], lo[:, :, 0:OWF], lo[:, :, C:C + OWF], op=MAX)
    tt(A[:, :, 0:OWF], t1[:, :, 0:OWF], lo[:, :, 2 * C:XWF], op=MAX)
    tt(c1[:, :, 0:OWF], hi[:, :, 0:OWF], hi[:, :, C:C + OWF], op=MIN)
    tt(Cm[:, :, 0:OWF], c1[:, :, 0:OWF], hi[:, :, 2 * C:XWF], op=MIN)
    p_, q_, r_, Bm = wt(), wt(), wt(), wt()
    tt(p_[:, :, 0:OWF], mid[:, :, 0:OWF], mid[:, :, C:C + OWF], op=MIN)
    tt(q_[:, :, 0:OWF], mid[:, :, 0:OWF], mid[:, :, C:C + OWF], op=MAX)
    tt(r_[:, :, 0:OWF], q_[:, :, 0:OWF], mid[:, :, 2 * C:XWF], op=MIN)
    tt(Bm[:, :, 0:OWF], p_[:, :, 0:OWF], r_[:, :, 0:OWF], op=MAX)
    s_, u_, v_ = wt(), wt(), wt()
    tt(s_[:, :, 0:OWF], A[:, :, 0:OWF], Bm[:, :, 0:OWF], op=MIN)
    tt(u_[:, :, 0:OWF], A[:, :, 0:OWF], Bm[:, :, 0:OWF], op=MAX)
    tt(v_[:, :, 0:OWF], u_[:, :, 0:OWF], Cm[:, :, 0:OWF], op=MIN)
    O = pools["o"].tile([P, 4, OWF], F16, tag="o", name="O")
    tt(O[:], s_[:, :, 0:OWF], v_[:, :, 0:OWF], op=MAX)

    # out: two column halves, un-permuting rows [0,2,1,3]; DMAs on SP
    yv = y[img].rearrange("(p j) w -> p j w", j=4)
    for hb in range(2):
        cb = hb * 768
        ot = pools["ostag"].tile([P, 4, 768], F32, tag="ostag", name="ot")
        nc.scalar.copy(ot[:, 0:3:2, :], O[:, 0:2, cb:cb + 768])
        nc.scalar.copy(ot[:, 1:4:2, :], O[:, 2:4, cb:cb + 768])
        nc.sync.dma_start(yv[:, :, cb:cb + 768], ot[:])


def build_median_nc(reps=1, n_imgs=IMGS_PER_CORE, split=None, layout=None):
    import os
    if layout is None:
        layout = os.environ.get("KLAYOUT", "rl4p")
    """layout="rl4" (default): non-interleaved 4-rows-per-partition blocks.
    layout="v1": interleaved half-pair layout. split=<int>: v2 DVE|Pool
    column split (requires a toolchain whose walrus accepts Pool TT)."""
    nc = bass.Bass("TRN2")
    x = nc.dram_tensor("x", [IMGS_PER_CORE, H, WC], F32, kind="ExternalInput")
    y = nc.dram_tensor("out", [IMGS_PER_CORE, H, WC], F32, kind="ExternalOutput")
    from contextlib import ExitStack

    if split is not None:
        assert split % 6 == 0 and 0 < split < OW
        pool_spec = [
            ("stag", 4), ("x", 2), ("ppd", 2), ("wd", 8), ("od", 2),
            ("ppp", 2), ("wp", 8), ("op", 2), ("ostag", 4),
        ]
    elif layout == "zip":
        pool_spec = [
            ("stag", 4), ("x", 3), ("pp", 4), ("work", 12),
            ("o", 3), ("ostag", 2),
        ]
    elif layout == "fw":
        pool_spec = [
            ("x", 3), ("pp", 2), ("work", 8), ("o", 1), ("ostag", 2),
        ]
    elif layout in ("rl4e", "rl4p"):
        pool_spec = [
            ("stag", 4), ("x", 4), ("pp", 2), ("work", 8),
            ("o", 3), ("ostag", 2),
        ]
    elif layout in ("rl4", "rl4c", "rl4d", "nodve", "dve2x"):
        pool_spec = [
            ("stag", 4), ("x", 3), ("pp", 2), ("work", 8),
            ("o", 3), ("ostag", 2),
        ]
    else:
        pool_spec = [
            ("stag", 6), ("x", 2), ("pp", 2), ("work", 8),
            ("o", 2), ("ostag", 4),
        ]
    with _TileContext(nc) as tc, ExitStack() as es:
        pools = {
            name: es.enter_context(tc.tile_pool(name=name, bufs=bufs))
            for name, bufs in pool_spec
        }
        if layout == "fw":
            imgs = [im for _ in range(reps) for im in range(n_imgs)]
            loaded = []
            for i, im in enumerate(imgs):
                loaded.append(_fw_load(nc, pools, x, im))
                if i >= 1:
                    _fw_compute(nc, pools, y, imgs[i - 1], loaded[i - 1])
                    loaded[i - 1] = None
            _fw_compute(nc, pools, y, imgs[-1], loaded[-1])
        elif layout == "rl4p":
            PRE = 2
            blocks = [
                (img, blk)
                for _ in range(reps)
                for img in range(n_imgs)
                for blk in range(2)
            ]
            loaded = []
            for i, (img, blk) in enumerate(blocks):
                loaded.append(_rl4e_load(nc, pools, x, img, blk))
                if i >= PRE:
                    im, bl = blocks[i - PRE]
                    _rl4e_compute(nc, pools, y, im, bl, loaded[i - PRE])
                    loaded[i - PRE] = None
            for i in range(len(blocks) - PRE, len(blocks)):
                im, bl = blocks[i]
                _rl4e_compute(nc, pools, y, im, bl, loaded[i])
                loaded[i] = None
        for _ in range(reps if layout not in ("rl4p", "fw") else 0):
            for img in range(n_imgs):
                if layout == "zip":
                    _emit_image_zip(nc, pools, x, y, img)
                    continue
                for blk in range(2):
                    if split is not None:
                        _emit_block_split(nc, pools, x, y, img, blk, split)
                    elif layout == "nodve":
                        _emit_block_rl4d(nc, pools, x, y, img, blk, dve_reps=0)
                    elif layout == "dve2x":
                        _emit_block_rl4d(nc, pools, x, y, img, blk, dve_reps=2)
                    elif layout == "rl4e":
                        _emit_block_rl4e(nc, pools, x, y, img, blk)
                    elif layout == "rl4d":
                        _emit_block_rl4d(nc, pools, x, y, img, blk)
                    elif layout == "rl4c":
                        _emit_block_rl4c(nc, pools, x, y, img, blk)
                    elif layout == "rl4":
                        _emit_block_rl4(nc, pools, x, y, img, blk)
                    else:
                        _emit_block(nc, pools, x, y, img, blk)
    _split_multi_waits(nc)
    return nc


_NC_CACHE = {}


def kernel(input_batch: np.ndarray) -> np.ndarray:
    input_batch = np.asarray(input_batch)
    assert input_batch.shape == (B, H, W, C), input_batch.shape
    xs = np.ascontiguousarray(input_batch.astype(np.float32, copy=False))
    xs = xs.reshape(B, H, WC)
    if "nc" not in _NC_CACHE:
        _NC_CACHE["nc"] = build_median_nc()
    nc = _NC_CACHE["nc"]
    in_maps = [
        {"x": xs[c * IMGS_PER_CORE:(c + 1) * IMGS_PER_CORE]} for c in range(N_CORES)
    ]
    res = run_bass_kernel_spmd(nc, in_maps, core_ids=list(range(N_CORES)))
    out = np.concatenate([res.results[c]["out"] for c in range(N_CORES)], axis=0)
    return out.reshape(B, H, W, C).astype(np.float32, copy=False)



# revision 46
# speedup vs baseline: 1.1722x; 1.1722x over previous
"""3x3 median blur (replicate borders) on 8 TRN2 NeuronCores.

Input : input_batch (32, 512, 512, 3) float32
Output: (32, 512, 512, 3) float32, per-channel 3x3 median, edge-replicated.

Strategy (all numbers measured on HW, not the cost model)
---------------------------------------------------------
Pure data parallel: 4 whole images per core; per image 2 column blocks of
256 output px; partition p holds rows 4p..4p+3 (layout "rl4e").

1. The original kernel was DMA-QUEUE-bound, not compute-bound: every
   dma_start issued from one engine shares that engine's single DGE queue,
   and one queue saturates at ~95 GB/s (41 us per 3.9 MB block = the whole
   block time). Spreading the same traffic over the three available queues
   (SP HWDGE / Act HWDGE / Pool SWDGE) runs the DMA-only pipeline at
   ~4.9 us per block. Queue split used here:
     * SP:   main rows 4p..4p+3, one overlapping-AP f32 load per block
     * Pool (gpsimd SWDGE): halo rows 4p-1 / 4p+4, as CASTING DMAs
       (f32 HBM -> bf16 SBUF, SWDGE-only feature) written directly into
       the bf16 X tile — removes those rows from the Act cast entirely
     * Act:  output DMA (f32 results)
2. DVE tensor_tensor bf16 streams at ~0.3-0.5 ns/elem/partition depending
   on AP shape; per-instruction cost is dominated by shape, not element
   count: contiguous [4,768] taps ~0.41 ns/elem, slot-strided [2,774]
   ~0.29, but 4D grouped-pixel APs (the "even-odd pair" trick) ~0.8+.
   So the network below minimizes *instruction cost*, not elem-ops:
     * vertical sort3 (5.0 ops/elem): sorted pairs of rows (4p,4p+1),
       (4p+2,4p+3) shared by two windows each; outer row inserted via
       lo=min(o,pmin), hi=max(o,pmax), mid=min(max(o,pmin),pmax). Output
       rows land in order [0,2,1,3]; the f32 out-cast un-permutes free.
     * horizontal: plain contiguous 3-tap max3/min3/med3 + final med3
       (med9 = med3(max3(lo), med3(mid), min3(hi))), all [4,768] ops.
   Pool engine cannot help: NeuronCore-v3 ISA rejects TensorTensor on it.
3. bf16 is safe: values in [0,255), median is an order statistic, so the
   result is an input value rounded to bf16 (abs err <= 0.5, rel ~2e-3).

History: 341 us (baseline, single DMA queue) -> ~225-230 us (this file).
Plateau analysis: the isolated DVE chain measures 11.4 us/block and the
no-compute pipeline floor 16.5 us/block; the kernel runs at their SUM
(~28 us/block) across every buffering / queue / emission-order /
lookahead variant tried, i.e. compute and the load-store pipeline do not
overlap under this Tile scheduler. Measured dead ends: SWDGE casting
stores (273 us), DVE f32-out finals (248 us), zipped L/R chains, deeper
lookahead, clamp relocation — all flat or worse.

MEASURED DEAD END (layout "fw" in this file): full-width blocks
(4 per core instead of 8) with all-casting SWDGE input = 267 us, i.e.
WORSE than the 228 us half-width default. The SBUF constraint forces all
input onto the Pool casting queue, whose transfers are the slow path
(consistent with the 273 us casting-store result); the halved per-block
fixed cost does not compensate. A full-width variant with fast-path f32
staging does not fit SBUF (needs ~266 KB of 208). The half-width rl4p
layout in this file is the optimum of the explored design space.
"""

import numpy as np

import concourse.bass as bass
import concourse.mybir as mybir
from concourse.tile import TileContext
from concourse.vector_clock import ScopedClock
from concourse.bass_utils import run_bass_kernel_spmd

F32 = mybir.dt.float32
# bf16, not fp16: measured on HW, DVE tensor_tensor streams ~1.0 cyc/elem in
# bf16 vs ~1.16 fp16 / ~1.9 fp32 / ~2.0 u8 (no 2x 16-bit mode exists on this
# hardware path, contrary to the cost model). bf16's 2^-8 relative step on
# values in [0,255) keeps the order-statistic result within ~4e-3 rel err.
F16 = mybir.dt.bfloat16
MIN = mybir.AluOpType.min
MAX = mybir.AluOpType.max

N_CORES = 8
B, H, W, C = 32, 512, 512, 3
WC = W * C                      # 1536 f32 elements per image row
IMGS_PER_CORE = B // N_CORES    # 4
HH = H // 2                     # rows per half (256)
P = 128                         # SBUF partitions
RL = HH // P                    # logical rows per partition (2)
SPX = W // 2 + 2                # stored pixels per column block (258)
XW = SPX * C * 2                # X tile width, fp16 interleaved (1548)
LW = XW - 6                     # sliding-pair width (1542)
OW = XW - 12                    # output width per block, interleaved (1536)
OWH = OW // 2                   # output f32 elems per half-row block (768)
INW = (SPX - 1) * C             # f32 elems loaded per row per block (771)


class _TileContext(TileContext):
    """TileContext whose final drain splits its semaphore waits.

    The stock TileContext attaches every end-of-kernel semaphore wait to a
    single Drain instruction; walrus' CTRL encoding fits only one sync wait
    per instruction, so kernels touching more than one processor fail to
    compile. Carry the waits on a chain of nops (one wait each) instead.
    """

    def _drain_and_barrier(self, tick_clock, wait_clock):
        carrier = self.nc.sync.nop(nofuse=True, hint="drain_wait_carrier")
        wait_clock.add_sem_waits(
            carrier.ins, ScopedClock({None: tick_clock.global_clock})
        )
        si = carrier.ins.sync_info
        waits = list(si.on_wait) if si and si.on_wait else []
        if len(waits) > 1:
            si.on_wait = waits[:1]
            for k in range(1, len(waits)):
                extra = self.nc.sync.nop(nofuse=True, hint=f"dwc{k}")
                extra.ins.sync_info = mybir.SyncInfo(
                    on_wait=[waits[k]], on_update=[]
                )
        self.nc.sync.drain()
        self.nc.all_engine_barrier()
        popped = self.nc._tile_sem_poison_stack.pop()
        assert popped is self._sem_poison
        self.nc.clear_and_free_semaphores(list(self.sems.allocated().values()))
        self.nc.all_engine_barrier()


def _split_multi_waits(nc):
    """Walrus in this toolchain encodes at most ONE sync wait per instruction.

    Tile attaches every needed semaphore wait directly to the consuming
    instruction; hoist all but the last onto standalone EventSemaphore
    instructions on the same engine immediately before it.
    """
    for f in nc.m.functions:
        for b in f.blocks:
            il = b.instructions
            out, changed = [], False
            for inst in il:
                si = inst.sync_info
                waits = list(si.on_wait) if si is not None and si.on_wait else []
                if len(waits) > 1:
                    changed = True
                    for w in waits[:-1]:
                        ev = mybir.InstEventSemaphore(
                            name=f"EVW-{nc.next_id()}",
                            engine=inst.engine,
                            ins=[],
                            outs=[],
                            sync_info=mybir.SyncInfo(on_wait=[w], on_update=[]),
                        )
                        out.append(ev)
                    si.on_wait = waits[-1:]
                out.append(inst)
            if changed:
                b.instructions = out


def _emit_block_split(nc, pools, x, y, img, blk, cs):
    """One (image, column-block) pass, columns split DVE | Pool engine.

    DVE computes output columns [0:cs), the Pool engine (gpsimd) computes
    [cs:OW) — independent slices of the same 18-op min/max chain, sized so
    both engines finish together (DVE fp16 runs 2x, Pool 1x at 1.2 GHz).
    `cs` must be a multiple of 6 (pixel granularity in the interleaved
    stream).
    """
    ttv = nc.vector.tensor_tensor
    ttp = nc.gpsimd.tensor_tensor
    c0 = 0 if blk == 0 else (W // 2 - 1) * C
    co = 0 if blk == 0 else OWH

    # ---- DMA in + cast/interleave: identical to the v1 block ----
    stag_t = pools["stag"].tile([P, 4, INW], F32, tag="stag")
    stag_b = pools["stag"].tile([P, 4, INW], F32, tag="stag")
    xi = x[img]
    ce = c0 + INW
    for j in range(3):
        nc.sync.dma_start(stag_t[:, 1 + j, :], xi[j:j + 2 * P - 1:2, c0:ce])
        nc.sync.dma_start(
            stag_b[:, j, :], xi[HH - 1 + j:HH - 1 + j + 2 * P - 1:2, c0:ce]
        )
    nc.sync.dma_start(stag_t[1:P, 0, :], xi[1:2 * P - 2:2, c0:ce])
    nc.sync.dma_start(stag_t[0:1, 0, :], xi[0:1, c0:ce])
    nc.sync.dma_start(stag_b[0:P - 1, 3, :], xi[HH + 2:H - 1:2, c0:ce])
    nc.sync.dma_start(stag_b[P - 1:P, 3, :], xi[H - 1:H, c0:ce])

    X = pools["x"].tile([P, 4, XW], F16, tag="x")
    if blk == 0:
        nc.scalar.copy(X[:, :, 6:XW:2], stag_t[:, :, :])
        nc.scalar.copy(X[:, :, 7:XW:2], stag_b[:, :, :])
        nc.scalar.copy(X[:, :, 0:6], X[:, :, 6:12])
    else:
        nc.scalar.copy(X[:, :, 0:LW:2], stag_t[:, :, :])
        nc.scalar.copy(X[:, :, 1:LW:2], stag_b[:, :, :])
        nc.scalar.copy(X[:, :, LW:XW], X[:, :, OW:LW])

    # ---- per-engine column slices of the med9 chain ----
    def emit_chain(tt, wpool, ppool, opool, lo_w, out_w, xoff):
        """Full vertical+horizontal chain on X[:, :, xoff : xoff+lo_w]."""

        def wt(w):
            return wpool.tile([P, RL, w], F16, tag=wpool.name, name="w")

        xs = X[:, :, xoff:xoff + lo_w]
        pmin = ppool.tile([P, RL, lo_w], F16, tag=ppool.name)
        pmax = ppool.tile([P, RL, lo_w], F16, tag=ppool.name)
        tt(pmin[:], xs[:, 0:2, :], xs[:, 1:3, :], op=MIN)
        tt(pmax[:], xs[:, 0:2, :], xs[:, 1:3, :], op=MAX)
        lo, tq, hi, mid = wt(lo_w), wt(lo_w), wt(lo_w), wt(lo_w)
        tt(lo[:], pmin[:], xs[:, 2:4, :], op=MIN)
        tt(tq[:], pmax[:], xs[:, 2:4, :], op=MIN)
        tt(hi[:], pmax[:], xs[:, 2:4, :], op=MAX)
        tt(mid[:], pmin[:], tq[:], op=MAX)

        t1, c1, p_, q_ = wt(out_w), wt(out_w), wt(out_w), wt(out_w)
        tt(t1[:], lo[:, :, 0:out_w], lo[:, :, 6:6 + out_w], op=MAX)
        tt(c1[:], hi[:, :, 0:out_w], hi[:, :, 6:6 + out_w], op=MIN)
        tt(p_[:], mid[:, :, 0:out_w], mid[:, :, 6:6 + out_w], op=MIN)
        tt(q_[:], mid[:, :, 0:out_w], mid[:, :, 6:6 + out_w], op=MAX)
        A, Cm, r_ = wt(out_w), wt(out_w), wt(out_w)
        tt(A[:], t1[:], lo[:, :, 12:12 + out_w], op=MAX)
        tt(Cm[:], c1[:], hi[:, :, 12:12 + out_w], op=MIN)
        tt(r_[:], q_[:], mid[:, :, 12:12 + out_w], op=MIN)
        Bm = wt(out_w)
        tt(Bm[:], p_[:], r_[:], op=MAX)
        s_, u_, v_ = wt(out_w), wt(out_w), wt(out_w)
        tt(s_[:], A[:], Bm[:], op=MIN)
        tt(u_[:], A[:], Bm[:], op=MAX)
        tt(v_[:], u_[:], Cm[:], op=MIN)
        O = opool.tile([P, RL, out_w], F16, tag=opool.name)
        tt(O[:], s_[:], v_[:], op=MAX)
        return O

    Od = emit_chain(ttv, pools["wd"], pools["ppd"], pools["od"], cs + 12, cs, 0)
    Op = emit_chain(ttp, pools["wp"], pools["ppp"], pools["op"], XW - cs, OW - cs, cs)

    # ---- de-interleave cast back to f32 and DMA out ----
    ot = pools["ostag"].tile([P, RL, OWH], F32, tag="ostag")
    ob = pools["ostag"].tile([P, RL, OWH], F32, tag="ostag")
    ch = cs // 2
    nc.scalar.copy(ot[:, :, 0:ch], Od[:, :, 0:cs:2])
    nc.scalar.copy(ob[:, :, 0:ch], Od[:, :, 1:cs:2])
    nc.scalar.copy(ot[:, :, ch:OWH], Op[:, :, 0:OW - cs:2])
    nc.scalar.copy(ob[:, :, ch:OWH], Op[:, :, 1:OW - cs:2])
    yt = y[img, 0:HH, :].rearrange("(p i) w -> p i w", i=RL)
    yb = y[img, HH:H, :].rearrange("(p i) w -> p i w", i=RL)
    nc.sync.dma_start(yt[:, :, co:co + OWH], ot[:])
    nc.sync.dma_start(yb[:, :, co:co + OWH], ob[:])


def _emit_block(nc, pools, x, y, img, blk):
    """One (image, column-block) pass: 256 output columns x 512 rows."""
    tt = nc.vector.tensor_tensor
    c0 = 0 if blk == 0 else (W // 2 - 1) * C          # f32 col offset (0 / 765)
    co = 0 if blk == 0 else OWH                       # out f32 col offset

    # ---- DMA in: per half, rows (2p-1 .. 2p+2) clamped at image edges ----
    stag_t = pools["stag"].tile([P, 4, INW], F32, tag="stag")
    stag_b = pools["stag"].tile([P, 4, INW], F32, tag="stag")
    xi = x[img]                                        # [H, WC]
    ce = c0 + INW
    # main rows as ONE overlapping-AP DMA per half: slot j holds row 2p+j
    # (top) / HH-1+2p+j (bottom) for j=0..2
    vt = xi.copy()
    vt.ap = mybir.VecI64Pair([[2 * WC, P], [WC, 3], [1, INW]])
    vt.offset = xi.offset + c0
    nc.sync.dma_start(stag_t[:, 1:4, :], vt)
    vb = xi.copy()
    vb.ap = mybir.VecI64Pair([[2 * WC, P], [WC, 3], [1, INW]])
    vb.offset = xi.offset + (HH - 1) * WC + c0
    nc.sync.dma_start(stag_b[:, 0:3, :], vb)
    # top halo: row 2p-1 (p>=1); p=0 clamps to row 0
    nc.sync.dma_start(stag_t[1:P, 0, :], xi[1:2 * P - 2:2, c0:ce])
    nc.sync.dma_start(stag_t[0:1, 0, :], xi[0:1, c0:ce])
    # bottom halo: row 256+2p+2 (p<=126); p=127 clamps to row 511
    nc.sync.dma_start(stag_b[0:P - 1, 3, :], xi[HH + 2:H - 1:2, c0:ce])
    nc.sync.dma_start(stag_b[P - 1:P, 3, :], xi[H - 1:H, c0:ce])

    # ---- cast f32 -> fp16, interleaving the two halves ----
    X = pools["x"].tile([P, 4, XW], F16, tag="x")
    if blk == 0:
        # pixels 0..256 at stored px 1..257; stored px 0 = left pad
        nc.scalar.copy(X[:, :, 6:XW:2], stag_t[:, :, :])
        nc.scalar.copy(X[:, :, 7:XW:2], stag_b[:, :, :])
        nc.scalar.copy(X[:, :, 0:6], X[:, :, 6:12])            # replicate pad
    else:
        # pixels 255..511 at stored px 0..256; stored px 257 = right pad
        nc.scalar.copy(X[:, :, 0:LW:2], stag_t[:, :, :])
        nc.scalar.copy(X[:, :, 1:LW:2], stag_b[:, :, :])
        nc.scalar.copy(X[:, :, LW:XW], X[:, :, OW:LW])         # replicate pad

    # ---- vertical pass: column sort3 with shared pairwise min/max ----
    def wt():
        return pools["work"].tile([P, RL, XW], F16, tag="work", name="work")

    # output row i's window is X slots (i, i+1, i+2); only pairs (0,1) and
    # (1,2) are consumed, so compute exactly those two pair-rows.
    pmin = pools["pp"].tile([P, RL, XW], F16, tag="pp")
    pmax = pools["pp"].tile([P, RL, XW], F16, tag="pp")
    tt(pmin[:], X[:, 0:2, :], X[:, 1:3, :], op=MIN)
    tt(pmax[:], X[:, 0:2, :], X[:, 1:3, :], op=MAX)
    lo, tq, hi, mid = wt(), wt(), wt(), wt()
    tt(lo[:], pmin[:], X[:, 2:4, :], op=MIN)           # min3
    tt(tq[:], pmax[:], X[:, 2:4, :], op=MIN)
    tt(hi[:], pmax[:], X[:, 2:4, :], op=MAX)           # max3
    tt(mid[:], pmin[:], tq[:], op=MAX)                 # med3

    # ---- horizontal pass (all shifts are even element offsets) ----
    # pair results are only consumed at [0:OW] (as index k-1 of a window
    # centered at k), so compute exactly that span.
    # max3(lo)/min3(hi) use the even-odd pair factorization: a half-width op
    # builds the shared pair m[j] = op(px[2j+1], px[2j+2]); two half-width
    # ops fold it into even/odd outputs -> 1.5 elem-ops/output instead of 2.
    def pxg(t, w=XW):
        return t[:, :, 0:w].rearrange("p r (g c) -> p r g c", c=6)

    GH = OW // 12                                     # output pixel pairs
    loG, hiG = pxg(lo), pxg(hi)
    mA, mC = wt(), wt()
    mAG, mCG = pxg(mA, GH * 6), pxg(mC, GH * 6)
    tt(mAG[:], loG[:, :, 1:2 * GH + 1:2, :], loG[:, :, 2:2 * GH + 2:2, :], op=MAX)
    tt(mCG[:], hiG[:, :, 1:2 * GH + 1:2, :], hiG[:, :, 2:2 * GH + 2:2, :], op=MIN)
    A, Cm = wt(), wt()
    AG, CG = pxg(A, OW), pxg(Cm, OW)
    tt(AG[:, :, 0:2 * GH:2, :], loG[:, :, 0:2 * GH:2, :], mAG[:], op=MAX)
    tt(AG[:, :, 1:2 * GH:2, :], mAG[:], loG[:, :, 3:2 * GH + 2:2, :], op=MAX)
    tt(CG[:, :, 0:2 * GH:2, :], hiG[:, :, 0:2 * GH:2, :], mCG[:], op=MIN)
    tt(CG[:, :, 1:2 * GH:2, :], mCG[:], hiG[:, :, 3:2 * GH + 2:2, :], op=MIN)
    p_, q_ = wt(), wt()
    tt(p_[:, :, 0:OW], mid[:, :, 0:OW], mid[:, :, 6:6 + OW], op=MIN)
    tt(q_[:, :, 0:OW], mid[:, :, 0:OW], mid[:, :, 6:6 + OW], op=MAX)
    r_ = wt()
    tt(r_[:, :, 0:OW], q_[:, :, 0:OW], mid[:, :, 12:XW], op=MIN)
    Bm = wt()
    tt(Bm[:, :, 0:OW], p_[:, :, 0:OW], r_[:, :, 0:OW], op=MAX)    # med3(mid)
    s_, u_, v_ = wt(), wt(), wt()
    tt(s_[:, :, 0:OW], A[:, :, 0:OW], Bm[:, :, 0:OW], op=MIN)
    tt(u_[:, :, 0:OW], A[:, :, 0:OW], Bm[:, :, 0:OW], op=MAX)
    tt(v_[:, :, 0:OW], u_[:, :, 0:OW], Cm[:, :, 0:OW], op=MIN)
    O = pools["o"].tile([P, RL, OW], F16, tag="o")
    tt(O[:], s_[:, :, 0:OW], v_[:, :, 0:OW], op=MAX)              # med9

    # ---- de-interleave cast back to f32 and DMA out ----
    ot = pools["ostag"].tile([P, RL, OWH], F32, tag="ostag")
    ob = pools["ostag"].tile([P, RL, OWH], F32, tag="ostag")
    nc.scalar.copy(ot[:], O[:, :, 0:OW:2])
    nc.scalar.copy(ob[:], O[:, :, 1:OW:2])
    yt = y[img, 0:HH, :].rearrange("(p i) w -> p i w", i=RL)
    yb = y[img, HH:H, :].rearrange("(p i) w -> p i w", i=RL)
    nc.sync.dma_start(yt[:, :, co:co + OWH], ot[:])
    nc.sync.dma_start(yb[:, :, co:co + OWH], ob[:])


def _emit_block_rl4(nc, pools, x, y, img, blk):
    """One (image, column-block) pass, non-interleaved RL=4 layout.

    Partition p holds image rows 4p..4p+3; staging has 6 row-slots
    (4p-1..4p+4, edge-clamped) so each input row is read from HBM 1.5x
    instead of the interleaved layout's 2x. bf16 needs no interleave
    (no 16-bit 2x DVE mode exists on HW), so pixels stay contiguous:
    1-px shifts are 3-element offsets and the output casts/DMA are
    contiguous.
    """
    tt = nc.vector.tensor_tensor
    R4 = 4                                             # rows per partition
    OW3 = W // 2 * C                                   # 768 out elems/row
    XW3 = OW3 + 2 * C                                  # 774 with halo px
    c0 = 0 if blk == 0 else (W // 2 - 1) * C
    co = 0 if blk == 0 else OW3
    ce = c0 + INW

    # ---- DMA in: slots 1..4 = rows 4p..4p+3 (each row once) ----
    stag = pools["stag"].tile([P, 6, INW], F32, tag="stag")
    xi = x[img]
    vm = xi.copy()
    vm.ap = mybir.VecI64Pair([[R4 * WC, P], [WC, R4], [1, INW]])
    vm.offset = xi.offset + c0
    nc.sync.dma_start(stag[:, 1:5, :], vm)
    # slot 0 = row 4p-1 (p>=1); p=0 clamps to row 0
    nc.sync.dma_start(stag[1:P, 0, :], xi[3:H - 4:R4, c0:ce])
    nc.sync.dma_start(stag[0:1, 0, :], xi[0:1, c0:ce])
    # slot 5 = row 4p+4 (p<=126); p=127 clamps to row 511
    nc.sync.dma_start(stag[0:P - 1, 5, :], xi[R4:H - 3:R4, c0:ce])
    nc.sync.dma_start(stag[P - 1:P, 5, :], xi[H - 1:H, c0:ce])

    # ---- cast f32 -> bf16 (contiguous) + 1-px replicate pad ----
    X = pools["x"].tile([P, 6, XW3], F16, tag="x")
    if blk == 0:
        nc.scalar.copy(X[:, :, C:XW3], stag[:, :, :])
        nc.scalar.copy(X[:, :, 0:C], X[:, :, C:2 * C])
    else:
        nc.scalar.copy(X[:, :, 0:INW], stag[:, :, :])
        nc.scalar.copy(X[:, :, INW:XW3], X[:, :, INW - C:INW])

    # ---- vertical pass: sort3 of rows with shared pairwise min/max ----
    def wt():
        return pools["work"].tile([P, R4, XW3], F16, tag="work", name="w")

    pmin = pools["pp"].tile([P, 5, XW3], F16, tag="pp")
    pmax = pools["pp"].tile([P, 5, XW3], F16, tag="pp")
    tt(pmin[:], X[:, 0:5, :], X[:, 1:6, :], op=MIN)
    tt(pmax[:], X[:, 0:5, :], X[:, 1:6, :], op=MAX)
    lo, tq, hi, mid = wt(), wt(), wt(), wt()
    tt(lo[:], pmin[:, 0:R4, :], X[:, 2:6, :], op=MIN)
    tt(tq[:], pmax[:, 0:R4, :], X[:, 2:6, :], op=MIN)
    tt(hi[:], pmax[:, 0:R4, :], X[:, 2:6, :], op=MAX)
    tt(mid[:], pmin[:, 0:R4, :], tq[:], op=MAX)

    # ---- horizontal pass (1-px shift = 3 elems) ----
    def pxg(t, w=XW3):
        return t[:, :, 0:w].rearrange("p r (g c) -> p r g c", c=C)

    GH = OW3 // (2 * C)                                # output pixel pairs
    loG, hiG = pxg(lo), pxg(hi)
    mA, mC = wt(), wt()
    mAG, mCG = pxg(mA, GH * C), pxg(mC, GH * C)
    tt(mAG[:], loG[:, :, 1:2 * GH + 1:2, :], loG[:, :, 2:2 * GH + 2:2, :], op=MAX)
    tt(mCG[:], hiG[:, :, 1:2 * GH + 1:2, :], hiG[:, :, 2:2 * GH + 2:2, :], op=MIN)
    A, Cm = wt(), wt()
    AG, CG = pxg(A, OW3), pxg(Cm, OW3)
    tt(AG[:, :, 0:2 * GH:2, :], loG[:, :, 0:2 * GH:2, :], mAG[:], op=MAX)
    tt(AG[:, :, 1:2 * GH:2, :], mAG[:], loG[:, :, 3:2 * GH + 2:2, :], op=MAX)
    tt(CG[:, :, 0:2 * GH:2, :], hiG[:, :, 0:2 * GH:2, :], mCG[:], op=MIN)
    tt(CG[:, :, 1:2 * GH:2, :], mCG[:], hiG[:, :, 3:2 * GH + 2:2, :], op=MIN)
    p_, q_ = wt(), wt()
    tt(p_[:, :, 0:OW3], mid[:, :, 0:OW3], mid[:, :, C:C + OW3], op=MIN)
    tt(q_[:, :, 0:OW3], mid[:, :, 0:OW3], mid[:, :, C:C + OW3], op=MAX)
    r_ = wt()
    tt(r_[:, :, 0:OW3], q_[:, :, 0:OW3], mid[:, :, 2 * C:2 * C + OW3], op=MIN)
    Bm = wt()
    tt(Bm[:, :, 0:OW3], p_[:, :, 0:OW3], r_[:, :, 0:OW3], op=MAX)
    s_, u_, v_ = wt(), wt(), wt()
    tt(s_[:, :, 0:OW3], A[:, :, 0:OW3], Bm[:, :, 0:OW3], op=MIN)
    tt(u_[:, :, 0:OW3], A[:, :, 0:OW3], Bm[:, :, 0:OW3], op=MAX)
    tt(v_[:, :, 0:OW3], u_[:, :, 0:OW3], Cm[:, :, 0:OW3], op=MIN)
    O = pools["o"].tile([P, R4, OW3], F16, tag="o")
    tt(O[:], s_[:, :, 0:OW3], v_[:, :, 0:OW3], op=MAX)            # med9

    # ---- cast back to f32 (contiguous) and one DMA out ----
    ot = pools["ostag"].tile([P, R4, OW3], F32, tag="ostag")
    nc.scalar.copy(ot[:], O[:])
    yv = y[img].rearrange("(p j) w -> p j w", j=R4)
    nc.sync.dma_start(yv[:, :, co:co + OW3], ot[:])


def _emit_block_rl4c(nc, pools, x, y, img, blk):
    """One (image, column-block) pass, RL=4 layout, 15.0 ops/elem network.

    Improvements over _emit_block_rl4 (17.55 ops/elem):
      * Vertical sort3 by inserting the outer row into a shared sorted pair:
        pairs (4p,4p+1),(4p+2,4p+3) are sorted once (pmin/pmax); each output
        row r folds its third ("outer") row in via
          lo3 = min(outer, pmin), hi3 = max(outer, pmax),
          mid3 = min(max(outer, pmin), pmax)
        -> 5.0 elem-ops per output instead of 6.5. Output rows land in
        permuted order [0,2,1,3] (even rows then odd rows of the quad) so
        both fold groups are single strided instructions; the output cast
        un-permutes for free with two strided Act copies.
      * Horizontal med3(mid) via the same shared-pair identity
        med3(a,b,c) = min(max(a, min(b,c)), max(b,c)) on even-odd pixel
        pairs -> 3.0 elem-ops instead of 4.0.
    """
    tt = nc.vector.tensor_tensor
    R4 = 4
    OW3 = W // 2 * C                                   # 768 out elems/row
    XW3 = OW3 + 2 * C                                  # 774 with halo px
    GH = OW3 // (2 * C)                                # 128 output pixel pairs
    c0 = 0 if blk == 0 else (W // 2 - 1) * C
    co = 0 if blk == 0 else OW3
    ce = c0 + INW

    # ---- DMA in: slots 1..4 = rows 4p..4p+3 (each row once) ----
    # Queue split (measured): the kernel is DMA-QUEUE-bound, not compute
    # bound. One HWDGE queue serving all DMAs caps at ~95 GB/s; spreading
    # main-in (SP) / halos (Pool SWDGE) / out (Act) runs at ~3.2x that.
    stag = pools["stag"].tile([P, 6, INW], F32, tag="stag")
    xi = x[img]
    vm = xi.copy()
    vm.ap = mybir.VecI64Pair([[R4 * WC, P], [WC, R4], [1, INW]])
    vm.offset = xi.offset + c0
    nc.sync.dma_start(stag[:, 1:5, :], vm)
    nc.gpsimd.dma_start(stag[1:P, 0, :], xi[3:H - 4:R4, c0:ce])
    nc.gpsimd.dma_start(stag[0:1, 0, :], xi[0:1, c0:ce])
    nc.gpsimd.dma_start(stag[0:P - 1, 5, :], xi[R4:H - 3:R4, c0:ce])
    nc.gpsimd.dma_start(stag[P - 1:P, 5, :], xi[H - 1:H, c0:ce])

    # ---- cast f32 -> bf16 (contiguous) + 1-px replicate pad ----
    X = pools["x"].tile([P, 6, XW3], F16, tag="x")
    if blk == 0:
        nc.scalar.copy(X[:, :, C:XW3], stag[:, :, :])
        nc.scalar.copy(X[:, :, 0:C], X[:, :, C:2 * C])
    else:
        nc.scalar.copy(X[:, :, 0:INW], stag[:, :, :])
        nc.scalar.copy(X[:, :, INW:XW3], X[:, :, INW - C:INW])

    def wt():
        return pools["work"].tile([P, R4, XW3], F16, tag="work", name="w")

    # ---- vertical: sorted pairs (slots 1,2) and (3,4), insert outer ----
    pmin = pools["pp"].tile([P, 2, XW3], F16, tag="pp")
    pmax = pools["pp"].tile([P, 2, XW3], F16, tag="pp")
    tt(pmin[:], X[:, 1:5:2, :], X[:, 2:6:2, :], op=MIN)
    tt(pmax[:], X[:, 1:5:2, :], X[:, 2:6:2, :], op=MAX)
    # output row order [4p, 4p+2, 4p+1, 4p+3]: evens use outer slots {0,2},
    # odds use outer slots {3,5}; both against pair rows {0,1}.
    lo, hi, t_, mid = wt(), wt(), wt(), wt()
    tt(lo[:, 0:2, :], X[:, 0:3:2, :], pmin[:], op=MIN)
    tt(lo[:, 2:4, :], X[:, 3:6:2, :], pmin[:], op=MIN)
    tt(hi[:, 0:2, :], X[:, 0:3:2, :], pmax[:], op=MAX)
    tt(hi[:, 2:4, :], X[:, 3:6:2, :], pmax[:], op=MAX)
    tt(t_[:, 0:2, :], X[:, 0:3:2, :], pmin[:], op=MAX)
    tt(t_[:, 2:4, :], X[:, 3:6:2, :], pmin[:], op=MAX)
    tt(mid[:, 0:2, :], t_[:, 0:2, :], pmax[:], op=MIN)
    tt(mid[:, 2:4, :], t_[:, 2:4, :], pmax[:], op=MIN)

    # ---- horizontal (1-px shift = 3 elems; even-odd pixel pairs) ----
    def pxg(t, w=XW3):
        return t[:, :, 0:w].rearrange("p r (g c) -> p r g c", c=C)

    loG, hiG, midG = pxg(lo), pxg(hi), pxg(mid)
    mA, mC, pm, pM = wt(), wt(), wt(), wt()
    mAG, mCG = pxg(mA, GH * C), pxg(mC, GH * C)
    pmG, pMG = pxg(pm, GH * C), pxg(pM, GH * C)
    tt(mAG[:], loG[:, :, 1:2 * GH + 1:2, :], loG[:, :, 2:2 * GH + 2:2, :], op=MAX)
    tt(mCG[:], hiG[:, :, 1:2 * GH + 1:2, :], hiG[:, :, 2:2 * GH + 2:2, :], op=MIN)
    tt(pmG[:], midG[:, :, 1:2 * GH + 1:2, :], midG[:, :, 2:2 * GH + 2:2, :], op=MIN)
    tt(pMG[:], midG[:, :, 1:2 * GH + 1:2, :], midG[:, :, 2:2 * GH + 2:2, :], op=MAX)
    A, Cm, tb, Bm = wt(), wt(), wt(), wt()
    AG, CG = pxg(A, OW3), pxg(Cm, OW3)
    tbG, BG = pxg(tb, OW3), pxg(Bm, OW3)
    tt(AG[:, :, 0:2 * GH:2, :], loG[:, :, 0:2 * GH:2, :], mAG[:], op=MAX)
    tt(AG[:, :, 1:2 * GH:2, :], mAG[:], loG[:, :, 3:2 * GH + 2:2, :], op=MAX)
    tt(CG[:, :, 0:2 * GH:2, :], hiG[:, :, 0:2 * GH:2, :], mCG[:], op=MIN)
    tt(CG[:, :, 1:2 * GH:2, :], mCG[:], hiG[:, :, 3:2 * GH + 2:2, :], op=MIN)
    # med3(mid) = min(max(outer, pm), pM)
    tt(tbG[:, :, 0:2 * GH:2, :], midG[:, :, 0:2 * GH:2, :], pmG[:], op=MAX)
    tt(tbG[:, :, 1:2 * GH:2, :], pmG[:], midG[:, :, 3:2 * GH + 2:2, :], op=MAX)
    tt(BG[:, :, 0:2 * GH:2, :], tbG[:, :, 0:2 * GH:2, :], pMG[:], op=MIN)
    tt(BG[:, :, 1:2 * GH:2, :], tbG[:, :, 1:2 * GH:2, :], pMG[:], op=MIN)
    # ---- final med3(A, Bm, Cm) ----
    s_, u_, v_ = wt(), wt(), wt()
    tt(s_[:, :, 0:OW3], A[:, :, 0:OW3], Bm[:, :, 0:OW3], op=MIN)
    tt(u_[:, :, 0:OW3], A[:, :, 0:OW3], Bm[:, :, 0:OW3], op=MAX)
    tt(v_[:, :, 0:OW3], u_[:, :, 0:OW3], Cm[:, :, 0:OW3], op=MIN)
    O = pools["o"].tile([P, R4, OW3], F16, tag="o")
    tt(O[:], s_[:, :, 0:OW3], v_[:, :, 0:OW3], op=MAX)            # med9

    # ---- cast back to f32, un-permuting rows [0,2,1,3] -> [0,1,2,3] ----
    ot = pools["ostag"].tile([P, R4, OW3], F32, tag="ostag")
    nc.scalar.copy(ot[:, 0:3:2, :], O[:, 0:2, :])
    nc.scalar.copy(ot[:, 1:4:2, :], O[:, 2:4, :])
    yv = y[img].rearrange("(p j) w -> p j w", j=R4)
    nc.scalar.dma_start(yv[:, :, co:co + OW3], ot[:])


def _emit_block_rl4d(nc, pools, x, y, img, blk, dve_reps=1):
    """One (image, column-block) pass optimized for DVE instruction cost.

    Measured reality: per-instruction cost is dominated by AP shape, not
    element count (contiguous [4,768] ~ 0.4 ns/elem; 4D grouped-pixel APs
    ~0.8 ns/elem + overhead). So: old-style contiguous vertical pass, and a
    plain 3-tap horizontal pass (19.1 elem-ops/output but only 18
    contiguous DVE instructions, no strided slots, no 4D APs).
    """
    tt = nc.vector.tensor_tensor
    R4 = 4
    OW3 = W // 2 * C                                   # 768 out elems/row
    XW3 = OW3 + 2 * C                                  # 774 with halo px
    c0 = 0 if blk == 0 else (W // 2 - 1) * C
    co = 0 if blk == 0 else OW3
    ce = c0 + INW

    # ---- DMA in (queue-split: main on SP, halos on Pool SWDGE) ----
    stag = pools["stag"].tile([P, 6, INW], F32, tag="stag")
    xi = x[img]
    vm = xi.copy()
    vm.ap = mybir.VecI64Pair([[R4 * WC, P], [WC, R4], [1, INW]])
    vm.offset = xi.offset + c0
    nc.sync.dma_start(stag[:, 1:5, :], vm)
    nc.gpsimd.dma_start(stag[1:P, 0, :], xi[3:H - 4:R4, c0:ce])
    nc.gpsimd.dma_start(stag[0:1, 0, :], xi[0:1, c0:ce])
    nc.gpsimd.dma_start(stag[0:P - 1, 5, :], xi[R4:H - 3:R4, c0:ce])
    nc.gpsimd.dma_start(stag[P - 1:P, 5, :], xi[H - 1:H, c0:ce])

    # ---- cast f32 -> bf16 (contiguous) + 1-px replicate pad ----
    X = pools["x"].tile([P, 6, XW3], F16, tag="x")
    if blk == 0:
        nc.scalar.copy(X[:, :, C:XW3], stag[:, :, :])
        nc.scalar.copy(X[:, :, 0:C], X[:, :, C:2 * C])
    else:
        nc.scalar.copy(X[:, :, 0:INW], stag[:, :, :])
        nc.scalar.copy(X[:, :, INW:XW3], X[:, :, INW - C:INW])

    def wt():
        return pools["work"].tile([P, R4, XW3], F16, tag="work", name="w")

    def med9_chain():
        # ---- vertical sort3, shared pairwise min/max, all contiguous ----
        pmin = pools["pp"].tile([P, 5, XW3], F16, tag="pp")
        pmax = pools["pp"].tile([P, 5, XW3], F16, tag="pp")
        tt(pmin[:], X[:, 0:5, :], X[:, 1:6, :], op=MIN)
        tt(pmax[:], X[:, 0:5, :], X[:, 1:6, :], op=MAX)
        lo, tq, hi, mid = wt(), wt(), wt(), wt()
        tt(lo[:], pmin[:, 0:R4, :], X[:, 2:6, :], op=MIN)
        tt(tq[:], pmax[:, 0:R4, :], X[:, 2:6, :], op=MIN)
        tt(hi[:], pmax[:, 0:R4, :], X[:, 2:6, :], op=MAX)
        tt(mid[:], pmin[:, 0:R4, :], tq[:], op=MAX)

        # ---- horizontal: plain 3-tap max3/min3/med3, contiguous [4,768] ----
        t1, A, c1, Cm = wt(), wt(), wt(), wt()
        tt(t1[:, :, 0:OW3], lo[:, :, 0:OW3], lo[:, :, C:C + OW3], op=MAX)
        tt(A[:, :, 0:OW3], t1[:, :, 0:OW3], lo[:, :, 2 * C:XW3], op=MAX)
        tt(c1[:, :, 0:OW3], hi[:, :, 0:OW3], hi[:, :, C:C + OW3], op=MIN)
        tt(Cm[:, :, 0:OW3], c1[:, :, 0:OW3], hi[:, :, 2 * C:XW3], op=MIN)
        p_, q_, r_, Bm = wt(), wt(), wt(), wt()
        tt(p_[:, :, 0:OW3], mid[:, :, 0:OW3], mid[:, :, C:C + OW3], op=MIN)
        tt(q_[:, :, 0:OW3], mid[:, :, 0:OW3], mid[:, :, C:C + OW3], op=MAX)
        tt(r_[:, :, 0:OW3], q_[:, :, 0:OW3], mid[:, :, 2 * C:XW3], op=MIN)
        tt(Bm[:, :, 0:OW3], p_[:, :, 0:OW3], r_[:, :, 0:OW3], op=MAX)
        s_, u_, v_ = wt(), wt(), wt()
        tt(s_[:, :, 0:OW3], A[:, :, 0:OW3], Bm[:, :, 0:OW3], op=MIN)
        tt(u_[:, :, 0:OW3], A[:, :, 0:OW3], Bm[:, :, 0:OW3], op=MAX)
        tt(v_[:, :, 0:OW3], u_[:, :, 0:OW3], Cm[:, :, 0:OW3], op=MIN)
        O = pools["o"].tile([P, R4, OW3], F16, tag="o")
        tt(O[:], s_[:, :, 0:OW3], v_[:, :, 0:OW3], op=MAX)        # med9
        return O

    if dve_reps == 0:
        O = pools["o"].tile([P, R4, OW3], F16, tag="o")
        tt(O[:], X[:, 0:4, 0:OW3], X[:, 1:5, 0:OW3], op=MIN)      # placeholder
    else:
        O = med9_chain()
        for _ in range(dve_reps - 1):
            O2 = med9_chain()
            O3 = pools["o"].tile([P, R4, OW3], F16, tag="o")
            tt(O3[:], O[:], O2[:], op=MIN)                        # == median
            O = O3

    # ---- cast back to f32 (contiguous) and one DMA out on Act queue ----
    ot = pools["ostag"].tile([P, R4, OW3], F32, tag="ostag")
    nc.scalar.copy(ot[:], O[:])
    yv = y[img].rearrange("(p j) w -> p j w", j=R4)
    nc.scalar.dma_start(yv[:, :, co:co + OW3], ot[:])


def _emit_block_rl4e(nc, pools, x, y, img, blk):
    """Best measured combination: rl4c's strided-slot vertical (insert into
    sorted pair, ~0.29 ns/elem) + rl4d's contiguous 3-tap horizontal
    (~0.41 ns/elem), DMA spread over all three queues with the 1-descriptor
    clamp DMAs kept off the Q7 SWDGE path (~1us fixed cost each there).
    """
    tt = nc.vector.tensor_tensor
    R4 = 4
    OW3 = W // 2 * C                                   # 768 out elems/row
    XW3 = OW3 + 2 * C                                  # 774 with halo px
    c0 = 0 if blk == 0 else (W // 2 - 1) * C
    co = 0 if blk == 0 else OW3
    ce = c0 + INW

    # ---- DMA in: SP main rows (f32 staging); halos as CASTING SWDGE DMAs
    # straight into X (bf16) on the Pool queue.
    stag = pools["stag"].tile([P, 4, INW], F32, tag="stag")
    X = pools["x"].tile([P, 6, XW3], F16, tag="x")
    xo = C if blk == 0 else 0                          # X col offset of px 0
    xi = x[img]
    vm = xi.copy()
    vm.ap = mybir.VecI64Pair([[R4 * WC, P], [WC, R4], [1, INW]])
    vm.offset = xi.offset + c0
    nc.sync.dma_start(stag[:], vm)
    nc.gpsimd.dma_start(X[1:P, 0, xo:xo + INW], xi[3:H - 4:R4, c0:ce])
    nc.gpsimd.dma_start(X[0:1, 0, xo:xo + INW], xi[0:1, c0:ce])
    nc.gpsimd.dma_start(X[0:P - 1, 5, xo:xo + INW], xi[R4:H - 3:R4, c0:ce])
    nc.gpsimd.dma_start(X[P - 1:P, 5, xo:xo + INW], xi[H - 1:H, c0:ce])

    # ---- cast main rows f32 -> bf16 + 1-px replicate pad ----
    if blk == 0:
        nc.scalar.copy(X[:, 1:5, C:XW3], stag[:])
        nc.scalar.copy(X[:, :, 0:C], X[:, :, C:2 * C])
    else:
        nc.scalar.copy(X[:, 1:5, 0:INW], stag[:])
        nc.scalar.copy(X[:, :, INW:XW3], X[:, :, INW - C:INW])

    def wt():
        return pools["work"].tile([P, R4, XW3], F16, tag="work", name="w")

    # ---- vertical: sorted pairs (slots 1,2),(3,4); insert outer row ----
    # output row order [4p, 4p+2, 4p+1, 4p+3] (evens then odds of the quad)
    pmin = pools["pp"].tile([P, 2, XW3], F16, tag="pp")
    pmax = pools["pp"].tile([P, 2, XW3], F16, tag="pp")
    tt(pmin[:], X[:, 1:5:2, :], X[:, 2:6:2, :], op=MIN)
    tt(pmax[:], X[:, 1:5:2, :], X[:, 2:6:2, :], op=MAX)
    lo, hi, t_, mid = wt(), wt(), wt(), wt()
    tt(lo[:, 0:2, :], X[:, 0:3:2, :], pmin[:], op=MIN)
    tt(lo[:, 2:4, :], X[:, 3:6:2, :], pmin[:], op=MIN)
    tt(hi[:, 0:2, :], X[:, 0:3:2, :], pmax[:], op=MAX)
    tt(hi[:, 2:4, :], X[:, 3:6:2, :], pmax[:], op=MAX)
    tt(t_[:, 0:2, :], X[:, 0:3:2, :], pmin[:], op=MAX)
    tt(t_[:, 2:4, :], X[:, 3:6:2, :], pmin[:], op=MAX)
    tt(mid[:, 0:2, :], t_[:, 0:2, :], pmax[:], op=MIN)
    tt(mid[:, 2:4, :], t_[:, 2:4, :], pmax[:], op=MIN)

    # ---- horizontal: plain 3-tap max3/min3/med3, contiguous [4,768] ----
    t1, A, c1, Cm = wt(), wt(), wt(), wt()
    tt(t1[:, :, 0:OW3], lo[:, :, 0:OW3], lo[:, :, C:C + OW3], op=MAX)
    tt(A[:, :, 0:OW3], t1[:, :, 0:OW3], lo[:, :, 2 * C:XW3], op=MAX)
    tt(c1[:, :, 0:OW3], hi[:, :, 0:OW3], hi[:, :, C:C + OW3], op=MIN)
    tt(Cm[:, :, 0:OW3], c1[:, :, 0:OW3], hi[:, :, 2 * C:XW3], op=MIN)
    p_, q_, r_, Bm = wt(), wt(), wt(), wt()
    tt(p_[:, :, 0:OW3], mid[:, :, 0:OW3], mid[:, :, C:C + OW3], op=MIN)
    tt(q_[:, :, 0:OW3], mid[:, :, 0:OW3], mid[:, :, C:C + OW3], op=MAX)
    tt(r_[:, :, 0:OW3], q_[:, :, 0:OW3], mid[:, :, 2 * C:XW3], op=MIN)
    tt(Bm[:, :, 0:OW3], p_[:, :, 0:OW3], r_[:, :, 0:OW3], op=MAX)
    s_, u_, v_ = wt(), wt(), wt()
    tt(s_[:, :, 0:OW3], A[:, :, 0:OW3], Bm[:, :, 0:OW3], op=MIN)
    tt(u_[:, :, 0:OW3], A[:, :, 0:OW3], Bm[:, :, 0:OW3], op=MAX)
    tt(v_[:, :, 0:OW3], u_[:, :, 0:OW3], Cm[:, :, 0:OW3], op=MIN)
    O = pools["o"].tile([P, R4, OW3], F16, tag="o")
    tt(O[:], s_[:, :, 0:OW3], v_[:, :, 0:OW3], op=MAX)            # med9

    # ---- cast back to f32 un-permuting rows; DMA out on Act queue ----
    ot = pools["ostag"].tile([P, R4, OW3], F32, tag="ostag")
    nc.scalar.copy(ot[:, 0:3:2, :], O[:, 0:2, :])
    nc.scalar.copy(ot[:, 1:4:2, :], O[:, 2:4, :])
    yv = y[img].rearrange("(p j) w -> p j w", j=R4)
    nc.scalar.dma_start(yv[:, :, co:co + OW3], ot[:])


def _emit_image_zip(nc, pools, x, y, img):
    """One whole image, the two column-block med9 chains interleaved.

    Consecutive DVE instructions alternate between the independent L/R
    block chains, hiding RAW latency (measured ~14% on back-to-back
    independent ops) and letting each block's loads/casts overlap the
    other block's compute.
    """
    tt = nc.vector.tensor_tensor
    R4 = 4
    OW3 = W // 2 * C
    XW3 = OW3 + 2 * C

    def load(blk):
        c0 = 0 if blk == 0 else (W // 2 - 1) * C
        ce = c0 + INW
        stag = pools["stag"].tile([P, 4, INW], F32, tag="stag")
        X = pools["x"].tile([P, 6, XW3], F16, tag="x")
        xo = C if blk == 0 else 0
        xi = x[img]
        vm = xi.copy()
        vm.ap = mybir.VecI64Pair([[R4 * WC, P], [WC, R4], [1, INW]])
        vm.offset = xi.offset + c0
        nc.sync.dma_start(stag[:], vm)
        nc.gpsimd.dma_start(X[1:P, 0, xo:xo + INW], xi[3:H - 4:R4, c0:ce])
        nc.gpsimd.dma_start(X[0:1, 0, xo:xo + INW], xi[0:1, c0:ce])
        nc.gpsimd.dma_start(X[0:P - 1, 5, xo:xo + INW], xi[R4:H - 3:R4, c0:ce])
        nc.gpsimd.dma_start(X[P - 1:P, 5, xo:xo + INW], xi[H - 1:H, c0:ce])
        if blk == 0:
            nc.scalar.copy(X[:, 1:5, C:XW3], stag[:])
            nc.scalar.copy(X[:, :, 0:C], X[:, :, C:2 * C])
        else:
            nc.scalar.copy(X[:, 1:5, 0:INW], stag[:])
            nc.scalar.copy(X[:, :, INW:XW3], X[:, :, INW - C:INW])
        return X

    def steps(X, blk):
        """DVE chain as a list of closures (one instruction each)."""
        co = 0 if blk == 0 else OW3
        st = {}

        def wt():
            return pools["work"].tile([P, R4, XW3], F16, tag="work", name="w")

        def s_vert():
            st["pmin"] = pools["pp"].tile([P, 2, XW3], F16, tag="pp", name="pmin")
            st["pmax"] = pools["pp"].tile([P, 2, XW3], F16, tag="pp", name="pmax")
            tt(st["pmin"][:], X[:, 1:5:2, :], X[:, 2:6:2, :], op=MIN)

        def fin():
            ot = pools["ostag"].tile([P, R4, OW3], F32, tag="ostag")
            nc.scalar.copy(ot[:, 0:3:2, :], st["O"][:, 0:2, :])
            nc.scalar.copy(ot[:, 1:4:2, :], st["O"][:, 2:4, :])
            yv = y[img].rearrange("(p j) w -> p j w", j=R4)
            nc.scalar.dma_start(yv[:, :, co:co + OW3], ot[:])

        seq = [s_vert,
               lambda: tt(st["pmax"][:], X[:, 1:5:2, :], X[:, 2:6:2, :], op=MAX)]
        for nm, args in [
            ("lo", (0, MIN, "pmin")), ("hi", (0, MAX, "pmax")),
            ("t_", (0, MAX, "pmin"))]:
            def fold(nm=nm, op=args[1], pair=args[2]):
                t = wt()
                st[nm] = t
                tt(t[:, 0:2, :], X[:, 0:3:2, :], st[pair][:], op=op)
                tt(t[:, 2:4, :], X[:, 3:6:2, :], st[pair][:], op=op)
            seq.append(fold)

        def mid():
            t = wt()
            st["mid"] = t
            tt(t[:, 0:2, :], st["t_"][:, 0:2, :], st["pmax"][:], op=MIN)
            tt(t[:, 2:4, :], st["t_"][:, 2:4, :], st["pmax"][:], op=MIN)
        seq.append(mid)

        def h(nm, a, ash, b, bsh, op):
            def go():
                t = wt()
                st[nm] = t
                src_a = st[a][:, :, ash:ash + OW3] if ash is not None else st[a][:, :, 0:OW3]
                src_b = st[b][:, :, bsh:bsh + OW3] if bsh is not None else st[b][:, :, 0:OW3]
                tt(t[:, :, 0:OW3], src_a, src_b, op=op)
            return go

        seq += [
            h("t1", "lo", 0, "lo", C, MAX),
            h("A", "t1", 0, "lo", 2 * C, MAX),
            h("c1", "hi", 0, "hi", C, MIN),
            h("Cm", "c1", 0, "hi", 2 * C, MIN),
            h("p_", "mid", 0, "mid", C, MIN),
            h("q_", "mid", 0, "mid", C, MAX),
            h("r_", "q_", 0, "mid", 2 * C, MIN),
            h("Bm", "p_", 0, "r_", 0, MAX),
            h("s_", "A", 0, "Bm", 0, MIN),
            h("u_", "A", 0, "Bm", 0, MAX),
            h("v_", "u_", 0, "Cm", 0, MIN),
        ]

        def last():
            O = pools["o"].tile([P, R4, OW3], F16, tag="o", name="O")
            st["O"] = O
            tt(O[:], st["s_"][:, :, 0:OW3], st["v_"][:, :, 0:OW3], op=MAX)
        seq.append(last)
        seq.append(fin)
        return seq

    XL = load(0)
    XR = load(1)
    sl, sr = steps(XL, 0), steps(XR, 1)
    for a, b in zip(sl, sr):
        a()
        b()


def _rl4e_load(nc, pools, x, img, blk):
    """Load+cast phase of an rl4e block: DMAs on SP/Pool queues + Act cast.

    Emitted AHEAD of earlier blocks' compute tails (software pipelining) so
    the in-order Act engine never has a future block's input cast queued
    behind an out-cast that waits on the DVE chain.
    """
    R4 = 4
    OW3 = W // 2 * C
    XW3 = OW3 + 2 * C
    c0 = 0 if blk == 0 else (W // 2 - 1) * C
    ce = c0 + INW
    stag = pools["stag"].tile([P, 4, INW], F32, tag="stag")
    X = pools["x"].tile([P, 6, XW3], F16, tag="x")
    xo = C if blk == 0 else 0
    xi = x[img]
    vm = xi.copy()
    vm.ap = mybir.VecI64Pair([[R4 * WC, P], [WC, R4], [1, INW]])
    vm.offset = xi.offset + c0
    nc.sync.dma_start(stag[:], vm)
    nc.gpsimd.dma_start(X[1:P, 0, xo:xo + INW], xi[3:H - 4:R4, c0:ce])
    nc.gpsimd.dma_start(X[0:P - 1, 5, xo:xo + INW], xi[R4:H - 3:R4, c0:ce])
    nc.gpsimd.dma_start(X[P - 1:P, 5, xo:xo + INW], xi[H - 1:H, c0:ce])
    # top clamp as a tiny Act copy (row 0 == slot1 of p=0, already cast);
    # Act accesses may not start at partition 127, so the bottom clamp
    # stays a 1-descriptor SWDGE DMA.
    if blk == 0:
        nc.scalar.copy(X[:, 1:5, C:XW3], stag[:])
        nc.scalar.copy(X[0:1, 0, C:XW3], X[0:1, 1, C:XW3])
        nc.scalar.copy(X[:, :, 0:C], X[:, :, C:2 * C])
    else:
        nc.scalar.copy(X[:, 1:5, 0:INW], stag[:])
        nc.scalar.copy(X[0:1, 0, 0:INW], X[0:1, 1, 0:INW])
        nc.scalar.copy(X[:, :, INW:XW3], X[:, :, INW - C:INW])
    return X


def _rl4e_compute(nc, pools, y, img, blk, X):
    """DVE chain + out-cast + out-DMA of an rl4e block."""
    tt = nc.vector.tensor_tensor
    R4 = 4
    OW3 = W // 2 * C
    XW3 = OW3 + 2 * C
    co = 0 if blk == 0 else OW3

    def wt():
        return pools["work"].tile([P, R4, XW3], F16, tag="work", name="w")

    pmin = pools["pp"].tile([P, 2, XW3], F16, tag="pp")
    pmax = pools["pp"].tile([P, 2, XW3], F16, tag="pp")
    tt(pmin[:], X[:, 1:5:2, :], X[:, 2:6:2, :], op=MIN)
    tt(pmax[:], X[:, 1:5:2, :], X[:, 2:6:2, :], op=MAX)
    # Folds as single 4-row instructions: outer rows {0,2,3,5} via nested
    # slicing; the sorted-pair operand broadcast to rows [0,1,0,1] with a
    # stride-0 middle dim (verified exact on HW).
    Xo = X[:, 0:6, :].rearrange("p (a b) w -> p a b w", a=2)[:, :, 0:3:2, :]

    def bcast(pt):
        v = pt[:, 0:2, :].copy()
        v.ap = mybir.VecI64Pair([[2 * XW3, P], [0, 2], [XW3, 2], [1, XW3]])
        return v

    pminB, pmaxB = bcast(pmin), bcast(pmax)
    lo, hi, t_, mid = wt(), wt(), wt(), wt()

    def r4(tl):
        return tl[:].rearrange("p (a b) w -> p a b w", a=2)

    tt(r4(lo), Xo, pminB, op=MIN)
    tt(r4(hi), Xo, pmaxB, op=MAX)
    tt(r4(t_), Xo, pminB, op=MAX)
    tt(r4(mid), r4(t_), pmaxB, op=MIN)

    t1, A, c1, Cm = wt(), wt(), wt(), wt()
    tt(t1[:, :, 0:OW3], lo[:, :, 0:OW3], lo[:, :, C:C + OW3], op=MAX)
    tt(A[:, :, 0:OW3], t1[:, :, 0:OW3], lo[:, :, 2 * C:XW3], op=MAX)
    tt(c1[:, :, 0:OW3], hi[:, :, 0:OW3], hi[:, :, C:C + OW3], op=MIN)
    tt(Cm[:, :, 0:OW3], c1[:, :, 0:OW3], hi[:, :, 2 * C:XW3], op=MIN)
    p_, q_, r_, Bm = wt(), wt(), wt(), wt()
    tt(p_[:, :, 0:OW3], mid[:, :, 0:OW3], mid[:, :, C:C + OW3], op=MIN)
    tt(q_[:, :, 0:OW3], mid[:, :, 0:OW3], mid[:, :, C:C + OW3], op=MAX)
    tt(r_[:, :, 0:OW3], q_[:, :, 0:OW3], mid[:, :, 2 * C:XW3], op=MIN)
    tt(Bm[:, :, 0:OW3], p_[:, :, 0:OW3], r_[:, :, 0:OW3], op=MAX)
    s_, u_, v_ = wt(), wt(), wt()
    tt(s_[:, :, 0:OW3], A[:, :, 0:OW3], Bm[:, :, 0:OW3], op=MIN)
    tt(u_[:, :, 0:OW3], A[:, :, 0:OW3], Bm[:, :, 0:OW3], op=MAX)
    tt(v_[:, :, 0:OW3], u_[:, :, 0:OW3], Cm[:, :, 0:OW3], op=MIN)
    O = pools["o"].tile([P, R4, OW3], F16, tag="o")
    tt(O[:], s_[:, :, 0:OW3], v_[:, :, 0:OW3], op=MAX)

    ot = pools["ostag"].tile([P, R4, OW3], F32, tag="ostag")
    nc.scalar.copy(ot[:, 0:3:2, :], O[:, 0:2, :])
    nc.scalar.copy(ot[:, 1:4:2, :], O[:, 2:4, :])
    yv = y[img].rearrange("(p j) w -> p j w", j=R4)
    nc.scalar.dma_start(yv[:, :, co:co + OW3], ot[:])


def _fw_load(nc, pools, x, img):
    """Full-width image load: all input via casting SWDGE DMAs into X."""
    XWF = WC + 2 * C                                   # 1542
    X = pools["x"].tile([P, 6, XWF], F16, tag="x")
    xi = x[img]
    vm = xi.copy()
    vm.ap = mybir.VecI64Pair([[4 * WC, P], [WC, 4], [1, WC]])
    vm.offset = xi.offset
    nc.gpsimd.dma_start(X[:, 1:5, C:C + WC], vm)
    nc.gpsimd.dma_start(X[1:P, 0, C:C + WC], xi[3:H - 4:4, :])
    nc.gpsimd.dma_start(X[0:P - 1, 5, C:C + WC], xi[4:H - 3:4, :])
    nc.gpsimd.dma_start(X[P - 1:P, 5, C:C + WC], xi[H - 1:H, :])
    nc.scalar.copy(X[0:1, 0, C:C + WC], X[0:1, 1, C:C + WC])   # top clamp
    nc.scalar.copy(X[:, :, 0:C], X[:, :, C:2 * C])             # left pad
    nc.scalar.copy(X[:, :, C + WC:XWF], X[:, :, WC:C + WC])    # right pad
    return X


def _fw_compute(nc, pools, y, img, X):
    """Full-width med9 chain + output; one image per pass."""
    tt = nc.vector.tensor_tensor
    OWF = WC                                           # 1536
    XWF = WC + 2 * C                                   # 1542

    def wt():
        return pools["work"].tile([P, 4, XWF], F16, tag="work", name="w")

    pmin = pools["pp"].tile([P, 2, XWF], F16, tag="pp", name="pmin")
    pmax = pools["pp"].tile([P, 2, XWF], F16, tag="pp", name="pmax")
    tt(pmin[:], X[:, 1:5:2, :], X[:, 2:6:2, :], op=MIN)
    tt(pmax[:], X[:, 1:5:2, :], X[:, 2:6:2, :], op=MAX)
    Xo = X[:, 0:6, :].rearrange("p (a b) w -> p a b w", a=2)[:, :, 0:3:2, :]

    def bcast(pt):
        v = pt[:, 0:2, :].copy()
        v.ap = mybir.VecI64Pair([[2 * XWF, P], [0, 2], [XWF, 2], [1, XWF]])
        return v

    pminB, pmaxB = bcast(pmin), bcast(pmax)
    lo, hi, t_, mid = wt(), wt(), wt(), wt()

    def r4(tl):
        return tl[:].rearrange("p (a b) w -> p a b w", a=2)

    tt(r4(lo), Xo, pminB, op=MIN)
    tt(r4(hi), Xo, pmaxB, op=MAX)
    tt(r4(t_), Xo, pminB, op=MAX)
    tt(r4(mid), r4(t_), pmaxB, op=MIN)

    t1, A, c1, Cm = wt(), wt(), wt(), wt()
    tt(t1[:, :, 0:OWF], lo[:, :, 0:OWF], lo[:, :, C:C + OWF], op=MAX)
    tt(A[:, :, 0:OWF], t1[:, :, 0:OWF], lo[:, :, 2 * C:XWF], op=MAX)
    tt(c1[:, :, 0:OWF], hi[:, :, 0:OWF], hi[:, :, C:C + OWF], op=MIN)
    tt(Cm[:, :, 0:OWF], c1[:, :, 0:OWF], hi[:, :, 2 * C:XWF], op=MIN)
    p_, q_, r_, Bm = wt(), wt(), wt(), wt()
    tt(p_[:, :, 0:OWF], mid[:, :, 0:OWF], mid[:, :, C:C + OWF], op=MIN)
    tt(q_[:, :, 0:OWF], mid[:, :, 0:OWF], mid[:, :, C:C + OWF], op=MAX)
    tt(r_[:, :, 0:OWF], q_[:, :, 0:OWF], mid[:, :, 2 * C:XWF], op=MIN)
    tt(Bm[:, :, 0:OWF], p_[:, :, 0:OWF], r_[:, :, 0:OWF], op=MAX)
    s_, u_, v_ = wt(), wt(), wt()
    tt(s_[:, :, 0:OWF], A[:, :, 0:OWF], Bm[:, :, 0:OWF], op=MIN)
    tt(u_[:, :, 0:OWF], A[:, :, 0:OWF], Bm[:, :, 0:OWF], op=MAX)
    tt(v_[:, :, 0:OWF], u_[:, :, 0:OWF], Cm[:, :, 0:OWF], op=MIN)
    O = pools["o"].tile([P, 4, OWF], F16, tag="o", name="O")
    tt(O[:], s_[:, :, 0:OWF], v_[:, :, 0:OWF], op=MAX)

    # out: two column halves, un-permuting rows [0,2,1,3]; DMAs on SP
    yv = y[img].rearrange("(p j) w -> p j w", j=4)
    for hb in range(2):
        cb = hb * 768
        ot = pools["ostag"].tile([P, 4, 768], F32, tag="ostag", name="ot")
        nc.scalar.copy(ot[:, 0:3:2, :], O[:, 0:2, cb:cb + 768])
        nc.scalar.copy(ot[:, 1:4:2, :], O[:, 2:4, cb:cb + 768])
        nc.sync.dma_start(yv[:, :, cb:cb + 768], ot[:])


def build_median_nc(reps=1, n_imgs=IMGS_PER_CORE, split=None, layout=None):
    import os
    if layout is None:
        layout = os.environ.get("KLAYOUT", "rl4p")
    """layout="rl4" (default): non-interleaved 4-rows-per-partition blocks.
    layout="v1": interleaved half-pair layout. split=<int>: v2 DVE|Pool
    column split (requires a toolchain whose walrus accepts Pool TT)."""
    nc = bass.Bass("TRN2")
    x = nc.dram_tensor("x", [IMGS_PER_CORE, H, WC], F32, kind="ExternalInput")
    y = nc.dram_tensor("out", [IMGS_PER_CORE, H, WC], F32, kind="ExternalOutput")
    from contextlib import ExitStack

    if split is not None:
        assert split % 6 == 0 and 0 < split < OW
        pool_spec = [
            ("stag", 4), ("x", 2), ("ppd", 2), ("wd", 8), ("od", 2),
            ("ppp", 2), ("wp", 8), ("op", 2), ("ostag", 4),
        ]
    elif layout == "zip":
        pool_spec = [
            ("stag", 4), ("x", 3), ("pp", 4), ("work", 12),
            ("o", 3), ("ostag", 2),
        ]
    elif layout == "fw":
        pool_spec = [
            ("x", 3), ("pp", 2), ("work", 8), ("o", 1), ("ostag", 2),
        ]
    elif layout in ("rl4e", "rl4p"):
        pool_spec = [
            ("stag", 4), ("x", 4), ("pp", 2), ("work", 8),
            ("o", 3), ("ostag", 2),
        ]
    elif layout in ("rl4", "rl4c", "rl4d", "nodve", "dve2x"):
        pool_spec = [
            ("stag", 4), ("x", 3), ("pp", 2), ("work", 8),
            ("o", 3), ("ostag", 2),
        ]
    else:
        pool_spec = [
            ("stag", 6), ("x", 2), ("pp", 2), ("work", 8),
            ("o", 2), ("ostag", 4),
        ]
    with _TileContext(nc) as tc, ExitStack() as es:
        pools = {
            name: es.enter_context(tc.tile_pool(name=name, bufs=bufs))
            for name, bufs in pool_spec
        }
        if layout == "fw":
            imgs = [im for _ in range(reps) for im in range(n_imgs)]
            loaded = []
            for i, im in enumerate(imgs):
                loaded.append(_fw_load(nc, pools, x, im))
                if i >= 1:
                    _fw_compute(nc, pools, y, imgs[i - 1], loaded[i - 1])
                    loaded[i - 1] = None
            _fw_compute(nc, pools, y, imgs[-1], loaded[-1])
        elif layout == "rl4p":
            PRE = 2
            blocks = [
                (img, blk)
                for _ in range(reps)
                for img in range(n_imgs)
                for blk in range(2)
            ]
            loaded = []
            for i, (img, blk) in enumerate(blocks):
                loaded.append(_rl4e_load(nc, pools, x, img, blk))
                if i >= PRE:
                    im, bl = blocks[i - PRE]
                    _rl4e_compute(nc, pools, y, im, bl, loaded[i - PRE])
                    loaded[i - PRE] = None
            for i in range(len(blocks) - PRE, len(blocks)):
                im, bl = blocks[i]
                _rl4e_compute(nc, pools, y, im, bl, loaded[i])
                loaded[i] = None
        for _ in range(reps if layout not in ("rl4p", "fw") else 0):
            for img in range(n_imgs):
                if layout == "zip":
                    _emit_image_zip(nc, pools, x, y, img)
                    continue
                for blk in range(2):
                    if split is not None:
                        _emit_block_split(nc, pools, x, y, img, blk, split)
                    elif layout == "nodve":
                        _emit_block_rl4d(nc, pools, x, y, img, blk, dve_reps=0)
                    elif layout == "dve2x":
                        _emit_block_rl4d(nc, pools, x, y, img, blk, dve_reps=2)
                    elif layout == "rl4e":
                        _emit_block_rl4e(nc, pools, x, y, img, blk)
                    elif layout == "rl4d":
                        _emit_block_rl4d(nc, pools, x, y, img, blk)
                    elif layout == "rl4c":
                        _emit_block_rl4c(nc, pools, x, y, img, blk)
                    elif layout == "rl4":
                        _emit_block_rl4(nc, pools, x, y, img, blk)
                    else:
                        _emit_block(nc, pools, x, y, img, blk)
    _split_multi_waits(nc)
    return nc


_NC_CACHE = {}


def kernel(input_batch: np.ndarray) -> np.ndarray:
    input_batch = np.asarray(input_batch)
    assert input_batch.shape == (B, H, W, C), input_batch.shape
    xs = np.ascontiguousarray(input_batch.astype(np.float32, copy=False))
    xs = xs.reshape(B, H, WC)
    if "nc" not in _NC_CACHE:
        _NC_CACHE["nc"] = build_median_nc()
    nc = _NC_CACHE["nc"]
    in_maps = [
        {"x": xs[c * IMGS_PER_CORE:(c + 1) * IMGS_PER_CORE]} for c in range(N_CORES)
    ]
    res = run_bass_kernel_spmd(nc, in_maps, core_ids=list(range(N_CORES)))
    out = np.concatenate([res.results[c]["out"] for c in range(N_CORES)], axis=0)
    return out.reshape(B, H, W, C).astype(np.float32, copy=False)

